# revision 28
# baseline (speedup 1.0000x reference)
"""CorrespondenceAttentionLayer on 8 TRN2 NeuronCores (Bass/Tile).

Strategy: row-shard the bs*L=4096 query rows into 8 shards of 512 (4 cores
per batch). No collectives; source-side K/V recomputed per core. Scores are
computed transposed (S^T[s,l]) so softmax normalization reduces over the
PSUM/matmul axis; masking is handled by a double AV matmul against
V*source_mask and V*(1-source_mask) with a ones-augmented column carrying
the softmax denominators. Matmuls run in float32r (1 cyc/row on PE).

kernel(**inputs) takes full unsharded inputs and returns the full output.
Host work is layout-only (transposes/slices).
"""
import os
import sys
from contextlib import ExitStack

import numpy as np

for _p in ("/opt/trn_rl_repo", "/root/.axon_site/_ro/trn_rl_repo"):
    if os.path.isdir(_p) and _p not in sys.path:
        sys.path.append(_p)

import concourse.bass as bass  # noqa: E402
import concourse.tile as tile  # noqa: E402
from concourse import bacc, mybir  # noqa: E402
from concourse.bass_utils import run_bass_kernel_spmd  # noqa: E402

BS, L, S, D = 2, 2048, 2048, 256
H, DH = 4, 64
NCORES = 8
LC = BS * L // NCORES      # 512 query rows per core
ST = S // 128              # 16 source tiles
F32 = mybir.dt.float32
F32R = mybir.dt.float32r
U8 = mybir.dt.uint8
AL = mybir.AluOpType
AF = mybir.ActivationFunctionType

_CACHED_NC = None


def build_nc():
    nc = bacc.Bacc("TRN2", target_bir_lowering=False, debug=False,
                   num_devices=NCORES)

    def din(name, shape, dt=F32):
        return nc.dram_tensor(name, shape, dt, kind="ExternalInput").ap()

    xT = din("xT", [D, LC])
    xpeT = din("xpeT", [D, LC])
    srcT = din("srcT", [D, S])
    srcpeT = din("srcpeT", [D, S])
    CT = din("CT", [S, LC])
    xm = din("xm", [LC], U8)
    sm = din("sm", [S], U8)
    Wq = din("Wq", [D, D])
    Wk = din("Wk", [D, D])
    Wv = din("Wv", [D, D])
    Wm = din("Wm", [D, D])
    W1 = din("W1", [2 * D, 2 * D])
    W2 = din("W2", [2 * D, D])
    g1 = din("g1", [D])
    b1 = din("b1", [D])
    g2 = din("g2", [D])
    b2 = din("b2", [D])
    outT = nc.dram_tensor("outT", [D, LC], F32, kind="ExternalOutput").ap()

    with tile.TileContext(nc) as tc, ExitStack() as ctx, \
            nc.allow_low_precision(reason="float32r matmul pipeline"):
        const = ctx.enter_context(tc.tile_pool(name="const", bufs=1))
        persist = ctx.enter_context(tc.tile_pool(name="persist", bufs=1))

        # ---------- constants ----------
        ones_f = const.tile([128, 128], F32)
        nc.vector.memset(ones_f[:], 1.0)
        ones_r = const.tile([128, 128], F32R)
        nc.vector.tensor_copy(ones_r[:], ones_f[:])
        i256_r = const.tile([128, 128], F32R)  # 1/256 everywhere
        nc.vector.tensor_scalar(out=i256_r[:], in0=ones_f[:], scalar1=1.0 / D,
                                scalar2=None, op0=AL.mult)
        negones_row = const.tile([1, LC], F32R)
        nc.vector.tensor_scalar(out=negones_row[:], in0=ones_f[0:1, 0:1],
                                scalar1=-1.0, scalar2=None, op0=AL.mult) \
            if False else None
        # build -1 row via memset+copy (memset on f32, copy to f32r)
        negones_f = const.tile([1, LC], F32)
        nc.vector.memset(negones_f[:], -1.0)
        nc.vector.tensor_copy(negones_row[:], negones_f[:])
        eps_t = const.tile([1, 1], F32)
        nc.vector.memset(eps_t[:], 1e-5)

        # ---------- weights (f32r, 1/8 folded into Wq) ----------
        wload = ctx.enter_context(tc.tile_pool(name="wload", bufs=1))

        def load_w(ap, kt, cols, scale=None):
            wf = wload.tile([128, kt, cols], F32, tag="wf")
            nc.sync.dma_start(out=wf[:], in_=ap.rearrange(
                "(t p) m -> p t m", p=128))
            wr = const.tile([128, kt, cols], F32R)
            if scale is None:
                nc.gpsimd.tensor_copy(wr[:], wf[:])
            else:
                nc.gpsimd.tensor_scalar(out=wr[:], in0=wf[:], scalar1=scale,
                                        scalar2=None, op0=AL.mult)
            return wr

        wq_r = load_w(Wq, 2, D, scale=1.0 / np.sqrt(DH))
        wk_r = load_w(Wk, 2, D)
        wv_r = load_w(Wv, 2, D)
        wm_r = load_w(Wm, 2, D)
        w1_r = load_w(W1, 4, 2 * D)
        w2_r = load_w(W2, 4, D)

        # LN params: per-partition cols [128,2] and f32r rows [1,256]
        def load_ln(gap, bap):
            gcol = const.tile([128, 2], F32)
            nc.sync.dma_start(out=gcol[:], in_=gap.rearrange("(t p) -> p t",
                                                             p=128))
            grow_f = const.tile([1, D], F32)
            nc.sync.dma_start(out=grow_f[:], in_=gap.rearrange(
                "(one d) -> one d", one=1))
            grow = const.tile([1, D], F32R)
            nc.vector.tensor_copy(grow[:], grow_f[:])
            brow_f = const.tile([1, D], F32)
            nc.sync.dma_start(out=brow_f[:], in_=bap.rearrange(
                "(one d) -> one d", one=1))
            brow = const.tile([1, D], F32R)
            nc.vector.tensor_copy(brow[:], brow_f[:])
            return gcol, grow, brow

        g1col, g1row, b1row = load_ln(g1, b1)
        g2col, g2row, b2row = load_ln(g2, b2)

        # ---------- masks ----------
        sm_u8 = const.tile([128, ST], U8)
        nc.sync.dma_start(out=sm_u8[:], in_=sm.rearrange("(i p) -> p i",
                                                         p=128))
        smf = const.tile([128, ST], F32)
        nc.vector.tensor_copy(smf[:], sm_u8[:])
        nsmf = const.tile([128, ST], F32)
        nc.vector.tensor_scalar(out=nsmf[:], in0=sm_u8[:], scalar1=-1.0,
                                scalar2=1.0, op0=AL.mult, op1=AL.add)
        xm_u8 = const.tile([1, LC], U8)
        nc.sync.dma_start(out=xm_u8[:], in_=xm.rearrange("(one l) -> one l",
                                                         one=1))
        nx_row = const.tile([1, LC], F32R)
        nc.vector.tensor_scalar(out=nx_row[:], in0=xm_u8[:], scalar1=-1.0,
                                scalar2=1.0, op0=AL.mult, op1=AL.add)

        # ---------- persistent activations ----------
        xT_f = persist.tile([128, 2, LC], F32)       # true x (final add)
        xr = persist.tile([128, 2, LC], F32R)        # x for mlp rhs
        QT = persist.tile([128, 2, LC], F32R)
        KT = persist.tile([128, 2, S], F32R)
        V1 = persist.tile([128, ST, H, DH + 1], F32R)
        V2 = persist.tile([128, ST, H, DH + 1], F32R)
        OT = persist.tile([128, 2, LC], F32R)
        nxb = persist.tile([DH + 1, LC], F32)
        ct_early = ctx.enter_context(tc.tile_pool(name="ct_early", bufs=1))
        cts_early = []
        for g in range(2):
            t = ct_early.tile([128, 2, LC], F32, name=f"cte_{g}", tag=f"cte{g}")
            nc.sync.dma_start(out=t[:], in_=CT[g * 256:(g + 1) * 256, :]
                              .rearrange("(j p) l -> p j l", p=128))
            cts_early.append(t)

        # ---------- phase 1: projections ----------
        with ExitStack() as c1:
            p1io = c1.enter_context(tc.tile_pool(name="p1io", bufs=1))
            p1ps = c1.enter_context(tc.tile_pool(name="p1ps", bufs=4,
                                                 space="PSUM"))
            xpe_f = p1io.tile([128, 2, LC], F32, tag="xio")
            nc.sync.dma_start(out=xT_f[:], in_=xT.rearrange(
                "(t p) l -> p t l", p=128))
            nc.sync.dma_start(out=xpe_f[:], in_=xpeT.rearrange(
                "(t p) l -> p t l", p=128))
            qin = p1io.tile([128, 2, LC], F32R, tag="qin")
            nc.vector.tensor_add(qin[:], xT_f[:], xpe_f[:])
            nc.vector.tensor_copy(xr[:], xT_f[:])

            # source loads in 512-col chunks; kin = src+srcpe, srcr = round(src)
            kin = p1io.tile([128, 2, S], F32R, tag="kin")
            srcr = p1io.tile([128, 2, S], F32R, tag="srcr")
            for nt in range(4):
                cs = slice(nt * 512, (nt + 1) * 512)
                sf = p1io.tile([128, 2, 512], F32, tag="sio", bufs=2,
                               name=f"sf_{nt}")
                pf = p1io.tile([128, 2, 512], F32, tag="pio", bufs=2,
                               name=f"pf_{nt}")
                nc.sync.dma_start(out=sf[:], in_=srcT[:, cs].rearrange(
                    "(t p) s -> p t s", p=128))
                nc.sync.dma_start(out=pf[:], in_=srcpeT[:, cs].rearrange(
                    "(t p) s -> p t s", p=128))
                nc.vector.tensor_add(kin[:, :, cs], sf[:], pf[:])
                nc.vector.tensor_copy(srcr[:, :, cs], sf[:])

            # Q^T [256, LC]
            for mt in range(2):
                ps = p1ps.tile([128, LC], F32, tag="qk", name=f"qps_{mt}")
                for kt in range(2):
                    nc.tensor.matmul(ps[:], wq_r[:, kt, mt * 128:(mt + 1) * 128],
                                     qin[:, kt, :], start=(kt == 0),
                                     stop=(kt == 1))
                nc.scalar.activation(QT[:, mt, :], ps[:], AF.Copy)
            # K^T [256, S]
            for mt in range(2):
                for nt in range(4):
                    ps = p1ps.tile([128, 512], F32, tag="qk",
                                   name=f"kps_{mt}_{nt}")
                    for kt in range(2):
                        nc.tensor.matmul(
                            ps[:], wk_r[:, kt, mt * 128:(mt + 1) * 128],
                            kin[:, kt, nt * 512:(nt + 1) * 512],
                            start=(kt == 0), stop=(kt == 1))
                    nc.scalar.activation(KT[:, mt, nt * 512:(nt + 1) * 512],
                                         ps[:], AF.Copy)
            # V [s, 256] -> vsb (ACT), V1 = V*sm (gpsimd TS), V2 = V - V1
            vsb = p1io.tile([128, ST, D], F32, tag="vsb")
            for i in range(ST):
                ps = p1ps.tile([128, D], F32, tag="vps", bufs=2,
                               name=f"vps_{i}")
                for kt in range(2):
                    nc.tensor.matmul(ps[:], srcr[:, kt, i * 128:(i + 1) * 128],
                                     wv_r[:, kt, :], start=(kt == 0),
                                     stop=(kt == 1))
                nc.scalar.activation(vsb[:, i, :], ps[:], AF.Copy)
            for i in range(ST):
                vsb_h = vsb[:, i, :].rearrange("p (h d) -> p h d", h=H)
                nc.vector.tensor_scalar(
                    out=V1[:, i, :, 0:DH], in0=vsb_h,
                    scalar1=smf[:, i:i + 1], scalar2=None, op0=AL.mult)
            nc.vector.tensor_sub(
                V2[:, :, :, 0:DH],
                vsb[:].rearrange("p i (h d) -> p i h d", h=H),
                V1[:, :, :, 0:DH])
            # sm / (1-sm) columns, broadcast over heads via 0-step AP
            sm_b = bass.AP(tensor=smf.tensor, offset=smf[:].offset,
                           ap=[smf[:].ap[0], smf[:].ap[1], [0, H], [0, 1]])
            nsm_b = bass.AP(tensor=nsmf.tensor, offset=nsmf[:].offset,
                            ap=[nsmf[:].ap[0], nsmf[:].ap[1], [0, H], [0, 1]])
            nc.vector.tensor_copy(V1[:, :, :, DH:DH + 1], sm_b)
            nc.vector.tensor_copy(V2[:, :, :, DH:DH + 1], nsm_b)

            # nxb [65, LC] via ones-matmul broadcast of nx_row
            psb = p1ps.tile([DH + 1, LC], F32, tag="nxbps", bufs=1)
            nc.tensor.matmul(psb[:], ones_r[0:1, 0:DH + 1], nx_row[:],
                             start=True, stop=True)
            nc.vector.tensor_copy(nxb[:], psb[:])

        # ---------- phase 2: attention ----------
        with ExitStack() as c2:
            sc_ps = c2.enter_context(tc.tile_pool(name="sc_ps", bufs=2,
                                                  space="PSUM"))
            av_ps = c2.enter_context(tc.tile_pool(name="av_ps", bufs=4,
                                                  space="PSUM"))
            p2sb = c2.enter_context(tc.tile_pool(name="p2sb", bufs=2))
            ct_pool = c2.enter_context(tc.tile_pool(name="ct", bufs=1))
            e_pool = c2.enter_context(tc.tile_pool(name="e", bufs=4))
            cts = list(cts_early)
            for g in range(2, 8):
                t = ct_pool.tile([128, 2, LC], F32, name=f"ct_{g}",
                                 tag=f"ct{g}")
                nc.sync.dma_start(out=t[:], in_=CT[g * 256:(g + 1) * 256, :]
                                  .rearrange("(j p) l -> p j l", p=128))
                cts.append(t)

            for hp in range(2):
                pv_ps = []
                for h2 in range(2):
                    pv_ps.append([av_ps.tile([DH + 1, LC], F32, tag="pv",
                                             name=f"pv_{hp}_{h2}_{v}")
                                  for v in range(2)])
                ehalf = {}
                for half in range(2):
                    for h2 in range(2):
                        ehalf[(half, h2)] = e_pool.tile(
                            [128, 8, LC], F32R, tag="eh",
                            name=f"eh_{hp}_{half}_{h2}")
                        ps_lo = h2 * DH
                        for g in range(4):
                            sc = sc_ps.tile([128, 2, LC], F32, tag="sc",
                                            name=f"sc_{hp}_{half}_{g}_{h2}")
                            for j in range(2):
                                i = half * 8 + g * 2 + j
                                nc.tensor.matmul(
                                    sc[:, j, :],
                                    KT[ps_lo:ps_lo + DH, hp,
                                       i * 128:(i + 1) * 128],
                                    QT[ps_lo:ps_lo + DH, hp, :],
                                    start=True, stop=True)
                            i0 = g * 2
                            nc.vector.tensor_mul(
                                ehalf[(half, h2)][:, i0:i0 + 2, :],
                                sc[:],
                                cts[half * 4 + g][:])
                    for h2 in range(2):
                        for ec in range(4):
                            eh_ap = ehalf[(half, h2)][:, ec * 2:ec * 2 + 2, :] \
                                .rearrange("p i l -> p (i l)")
                            nc.scalar.activation(eh_ap, eh_ap.bitcast(F32),
                                                 AF.Exp)
                        h = 2 * hp + h2
                        for j in range(8):
                            i = half * 8 + j
                            for v, pv in enumerate(pv_ps[h2]):
                                vt = (V1 if v == 0 else V2)
                                nc.tensor.matmul(
                                    pv[:], vt[:, i, h, :],
                                    ehalf[(half, h2)][:, j, :],
                                    start=(i == 0), stop=(i == ST - 1),
                                    skip_group_check=True)
                # combines
                for h2 in range(2):
                    h = 2 * hp + h2
                    p1t, p2t = pv_ps[h2]
                    tmp = p2sb.tile([DH + 1, LC], F32, tag="tmp")
                    nc.vector.tensor_mul(tmp[:], nxb[:], p2t[:])
                    ocomb = p2sb.tile([DH + 1, LC], F32, tag="oc")
                    nc.vector.tensor_add(ocomb[:], tmp[:], p1t[:])
                    rr = p2sb.tile([128, LC], F32R, tag="rr")
                    nc.vector.reciprocal(rr[DH:DH + 1, :],
                                         ocomb[DH:DH + 1, :])
                    rb = av_ps.tile([DH, LC], F32, tag="pv")
                    nc.tensor.matmul(rb[:], ones_r[DH:DH + 1, 0:DH],
                                     rr[DH:DH + 1, :], start=True, stop=True)
                    otmp = p2sb.tile([DH, LC], F32R, tag="ot")
                    nc.vector.scalar_tensor_tensor(
                        out=otmp[:], in0=ocomb[0:DH, :], scalar=1.0,
                        in1=rb[:], op0=AL.mult, op1=AL.mult)
                    nc.sync.dma_start(out=OT[h2 * DH:(h2 + 1) * DH, hp, :],
                                      in_=otmp[:])

        # ---------- phase 3: merge + LN1 + MLP + LN2 + residual ----------
        with ExitStack() as c3:
            p3ps = c3.enter_context(tc.tile_pool(name="p3ps", bufs=2,
                                                 space="PSUM"))
            p3sb = c3.enter_context(tc.tile_pool(name="p3sb", bufs=2))

            def layernorm_T(msb, gcol, grow, brow, out_r, out_f32=None,
                            add_x=False):
                """msb: [128,2,LC] f32r sbuf (features on partitions).
                Writes normalized result to out_r (f32r) or, if add_x,
                writes out_f32 = LN(msb) + xT_f."""
                msq = p3sb.tile([128, 2, LC], F32R, tag="msq")
                nc.gpsimd.tensor_mul(msq[:], msb[:], msb[:])
                mean_b = p3ps.tile([128, LC], F32, tag="lnps", bufs=4)
                for kt in range(2):
                    nc.tensor.matmul(mean_b[:], i256_r[:], msb[:, kt, :],
                                     start=(kt == 0), stop=(kt == 1))
                ssq = p3ps.tile([1, LC], F32, tag="lnrow", bufs=1)
                for kt in range(2):
                    nc.tensor.matmul(ssq[:], ones_r[:, 0:1], msq[:, kt, :],
                                     start=(kt == 0), stop=(kt == 1))
                mrow = p3sb.tile([1, LC], F32, tag="lnr1")
                nc.vector.tensor_copy(mrow[:], mean_b[0:1, :])
                m2 = p3sb.tile([1, LC], F32, tag="lnr2")
                nc.vector.tensor_mul(m2[:], mrow[:], mrow[:])
                varr = p3sb.tile([1, LC], F32, tag="lnr3")
                nc.vector.scalar_tensor_tensor(
                    out=varr[:], in0=ssq[:], scalar=1.0 / D, in1=m2[:],
                    op0=AL.mult, op1=AL.subtract)
                sd = p3sb.tile([1, LC], F32, tag="lnr4")
                nc.scalar.activation(sd[:], varr[:], AF.Sqrt, bias=eps_t[:])
                rstd = p3sb.tile([1, LC], F32R, tag="lnr5")
                nc.vector.reciprocal(rstd[:], sd[:])
                mr = p3sb.tile([1, LC], F32R, tag="lnr6")
                nc.vector.tensor_mul(mr[:], mrow[:], rstd[:].bitcast(F32))
                ab = p3ps.tile([128, LC], F32, tag="lnps", bufs=4)
                nc.tensor.matmul(ab[:], ones_r[0:1, :], rstd[:],
                                 start=True, stop=True)
                for t in range(2):
                    bfull = p3ps.tile([128, LC], F32, tag="lnps", bufs=4)
                    nc.tensor.matmul(bfull[:],
                                     grow[:, t * 128:(t + 1) * 128], mr[:],
                                     start=True, stop=False)
                    nc.tensor.matmul(bfull[:],
                                     brow[:, t * 128:(t + 1) * 128],
                                     negones_row[:], start=False, stop=True)
                    tmp = p3sb.tile([128, LC], F32, tag="lnt")
                    nc.vector.scalar_tensor_tensor(
                        out=tmp[:], in0=msb[:, t, :], scalar=gcol[:, t:t + 1],
                        in1=ab[:], op0=AL.mult, op1=AL.mult)
                    if add_x:
                        u = p3sb.tile([128, LC], F32, tag="lnu")
                        nc.vector.tensor_sub(u[:], tmp[:], bfull[:])
                        nc.gpsimd.tensor_add(out_f32[:, t, :], u[:],
                                             xT_f[:, t, :])
                    else:
                        nc.vector.tensor_sub(out_r[:, t, :], tmp[:],
                                             bfull[:])

            # merge: M^T = Wm^T @ OT
            msb = p3sb.tile([128, 2, LC], F32R, tag="msb")
            for mt in range(2):
                ps = p3ps.tile([128, LC], F32, tag="mm")
                for kt in range(2):
                    nc.tensor.matmul(ps[:], wm_r[:, kt, mt * 128:(mt + 1) * 128],
                                     OT[:, kt, :], start=(kt == 0),
                                     stop=(kt == 1))
                nc.scalar.activation(msb[:, mt, :], ps[:], AF.Copy)
            msg = p3sb.tile([128, 2, LC], F32R, tag="msg")
            layernorm_T(msb, g1col, g1row, b1row, msg)

            # mlp1 + relu
            relu = p3sb.tile([128, 4, LC], F32R, tag="relu")
            for ft in range(4):
                ps = p3ps.tile([128, LC], F32, tag="mm")
                for kt in range(4):
                    rhs = xr[:, kt, :] if kt < 2 else msg[:, kt - 2, :]
                    nc.tensor.matmul(ps[:],
                                     w1_r[:, kt, ft * 128:(ft + 1) * 128],
                                     rhs, start=(kt == 0), stop=(kt == 3))
                nc.scalar.activation(relu[:, ft, :], ps[:], AF.Relu)
            # mlp2
            m2sb = p3sb.tile([128, 2, LC], F32R, tag="m2sb")
            for mt in range(2):
                ps = p3ps.tile([128, LC], F32, tag="mm")
                for kt in range(4):
                    nc.tensor.matmul(ps[:],
                                     w2_r[:, kt, mt * 128:(mt + 1) * 128],
                                     relu[:, kt, :], start=(kt == 0),
                                     stop=(kt == 3))
                nc.scalar.activation(m2sb[:, mt, :], ps[:], AF.Copy)
            final = p3sb.tile([128, 2, LC], F32, tag="final")
            layernorm_T(m2sb, g2col, g2row, b2row, None, out_f32=final,
                        add_x=True)
            nc.sync.dma_start(out=outT.rearrange("(t p) l -> p t l", p=128),
                              in_=final[:])

    nc.compile()
    return nc


def _get_nc():
    global _CACHED_NC
    if _CACHED_NC is None:
        _CACHED_NC = build_nc()
    return _CACHED_NC


def kernel(x, source, x_pe, source_pe, x_mask, source_mask, compatibility,
           Wq, Wk, Wv, Wmerge, Wmlp1, Wmlp2, ln1_g, ln1_b, ln2_g, ln2_b):
    nc = _get_nc()
    f32 = np.float32
    xT = np.ascontiguousarray(np.asarray(x, f32).transpose(0, 2, 1))
    xpeT = np.ascontiguousarray(np.asarray(x_pe, f32).transpose(0, 2, 1))
    srcT = np.ascontiguousarray(np.asarray(source, f32).transpose(0, 2, 1))
    srcpeT = np.ascontiguousarray(
        np.asarray(source_pe, f32).transpose(0, 2, 1))
    CT = np.ascontiguousarray(
        np.asarray(compatibility, f32).transpose(0, 2, 1))
    xmu = np.asarray(x_mask).astype(np.uint8)
    smu = np.asarray(source_mask).astype(np.uint8)
    weights = {
        "Wq": np.asarray(Wq, f32), "Wk": np.asarray(Wk, f32),
        "Wv": np.asarray(Wv, f32), "Wm": np.asarray(Wmerge, f32),
        "W1": np.asarray(Wmlp1, f32), "W2": np.asarray(Wmlp2, f32),
        "g1": np.asarray(ln1_g, f32), "b1": np.asarray(ln1_b, f32),
        "g2": np.asarray(ln2_g, f32), "b2": np.asarray(ln2_b, f32),
    }
    in_maps = []
    for c in range(NCORES):
        b, l0 = c // (NCORES // BS), (c % (NCORES // BS)) * LC
        sl = slice(l0, l0 + LC)
        in_maps.append({
            "xT": np.ascontiguousarray(xT[b][:, sl]),
            "xpeT": np.ascontiguousarray(xpeT[b][:, sl]),
            "srcT": srcT[b],
            "srcpeT": srcpeT[b],
            "CT": np.ascontiguousarray(CT[b][:, sl]),
            "xm": np.ascontiguousarray(xmu[b][sl]),
            "sm": smu[b],
            **weights,
        })
    res = run_bass_kernel_spmd(nc, in_maps, list(range(NCORES)))
    outT = np.empty((BS, D, L), f32)
    for c in range(NCORES):
        b, l0 = c // (NCORES // BS), (c % (NCORES // BS)) * LC
        outT[b][:, l0:l0 + LC] = res.results[c]["outT"]
    return np.ascontiguousarray(outT.transpose(0, 2, 1))


# revision 34
# speedup vs baseline: 1.0032x; 1.0032x over previous
"""CorrespondenceAttentionLayer on 8 TRN2 NeuronCores (Bass/Tile).

Strategy: row-shard the bs*L=4096 query rows into 8 shards of 512 (4 cores
per batch). No collectives; source-side K/V recomputed per core. Scores are
computed transposed (S^T[s,l]) so softmax normalization reduces over the
PSUM/matmul axis; masking is handled by a double AV matmul against
V*source_mask and V*(1-source_mask) with a ones-augmented column carrying
the softmax denominators. Matmuls run in float32r (1 cyc/row on PE).

kernel(**inputs) takes full unsharded inputs and returns the full output.
Host work is layout-only (transposes/slices).
"""
import os
import sys
from contextlib import ExitStack

import numpy as np

for _p in ("/opt/trn_rl_repo", "/root/.axon_site/_ro/trn_rl_repo"):
    if os.path.isdir(_p) and _p not in sys.path:
        sys.path.append(_p)

import concourse.bass as bass  # noqa: E402
import concourse.tile as tile  # noqa: E402
from concourse import bacc, mybir  # noqa: E402
from concourse.bass_utils import run_bass_kernel_spmd  # noqa: E402

BS, L, S, D = 2, 2048, 2048, 256
H, DH = 4, 64
NCORES = 8
LC = BS * L // NCORES      # 512 query rows per core
ST = S // 128              # 16 source tiles
F32 = mybir.dt.float32
F32R = mybir.dt.float32r
U8 = mybir.dt.uint8
AL = mybir.AluOpType
AF = mybir.ActivationFunctionType

_CACHED_NC = None


def build_nc():
    nc = bacc.Bacc("TRN2", target_bir_lowering=False, debug=False,
                   num_devices=NCORES)

    def din(name, shape, dt=F32):
        return nc.dram_tensor(name, shape, dt, kind="ExternalInput").ap()

    xT = din("xT", [D, LC])
    xpeT = din("xpeT", [D, LC])
    srcT = din("srcT", [D, S])
    srcpeT = din("srcpeT", [D, S])
    CT = din("CT", [S, LC])
    xm = din("xm", [LC], U8)
    sm = din("sm", [S], U8)
    Wq = din("Wq", [D, D])
    Wk = din("Wk", [D, D])
    Wv = din("Wv", [D, D])
    Wm = din("Wm", [D, D])
    W1 = din("W1", [2 * D, 2 * D])
    W2 = din("W2", [2 * D, D])
    g1 = din("g1", [D])
    b1 = din("b1", [D])
    g2 = din("g2", [D])
    b2 = din("b2", [D])
    outT = nc.dram_tensor("outT", [D, LC], F32, kind="ExternalOutput").ap()

    with tile.TileContext(nc) as tc, ExitStack() as ctx, \
            nc.allow_low_precision(reason="float32r matmul pipeline"):
        const = ctx.enter_context(tc.tile_pool(name="const", bufs=1))
        persist = ctx.enter_context(tc.tile_pool(name="persist", bufs=1))

        # ---------- constants ----------
        ones_f = const.tile([128, 128], F32)
        nc.vector.memset(ones_f[:], 1.0)
        ones_r = const.tile([128, 128], F32R)
        nc.vector.tensor_copy(ones_r[:], ones_f[:])
        i256_r = const.tile([128, 128], F32R)  # 1/256 everywhere
        nc.vector.tensor_scalar(out=i256_r[:], in0=ones_f[:], scalar1=1.0 / D,
                                scalar2=None, op0=AL.mult)
        negones_row = const.tile([1, LC], F32R)
        nc.vector.tensor_scalar(out=negones_row[:], in0=ones_f[0:1, 0:1],
                                scalar1=-1.0, scalar2=None, op0=AL.mult) \
            if False else None
        # build -1 row via memset+copy (memset on f32, copy to f32r)
        negones_f = const.tile([1, LC], F32)
        nc.vector.memset(negones_f[:], -1.0)
        nc.vector.tensor_copy(negones_row[:], negones_f[:])
        eps_t = const.tile([1, 1], F32)
        nc.vector.memset(eps_t[:], 1e-5)

        # ---------- weights (f32r, 1/8 folded into Wq) ----------
        wload = ctx.enter_context(tc.tile_pool(name="wload", bufs=1))

        def load_w(ap, kt, cols, scale=None):
            wf = wload.tile([128, kt, cols], F32, tag="wf")
            nc.sync.dma_start(out=wf[:], in_=ap.rearrange(
                "(t p) m -> p t m", p=128))
            wr = const.tile([128, kt, cols], F32R)
            if scale is None:
                nc.gpsimd.tensor_copy(wr[:], wf[:])
            else:
                nc.gpsimd.tensor_scalar(out=wr[:], in0=wf[:], scalar1=scale,
                                        scalar2=None, op0=AL.mult)
            return wr

        wq_r = load_w(Wq, 2, D, scale=1.0 / np.sqrt(DH))
        wk_r = load_w(Wk, 2, D)
        wv_r = load_w(Wv, 2, D)
        wm_r = load_w(Wm, 2, D)
        w1_r = load_w(W1, 4, 2 * D)
        w2_r = load_w(W2, 4, D)

        # LN params: per-partition cols [128,2] and f32r rows [1,256]
        def load_ln(gap, bap):
            gcol = const.tile([128, 2], F32)
            nc.sync.dma_start(out=gcol[:], in_=gap.rearrange("(t p) -> p t",
                                                             p=128))
            grow_f = const.tile([1, D], F32)
            nc.sync.dma_start(out=grow_f[:], in_=gap.rearrange(
                "(one d) -> one d", one=1))
            grow = const.tile([1, D], F32R)
            nc.vector.tensor_copy(grow[:], grow_f[:])
            brow_f = const.tile([1, D], F32)
            nc.sync.dma_start(out=brow_f[:], in_=bap.rearrange(
                "(one d) -> one d", one=1))
            brow = const.tile([1, D], F32R)
            nc.vector.tensor_copy(brow[:], brow_f[:])
            return gcol, grow, brow

        g1col, g1row, b1row = load_ln(g1, b1)
        g2col, g2row, b2row = load_ln(g2, b2)

        # ---------- masks ----------
        sm_u8 = const.tile([128, ST], U8)
        nc.sync.dma_start(out=sm_u8[:], in_=sm.rearrange("(i p) -> p i",
                                                         p=128))
        smf = const.tile([128, ST], F32)
        nc.vector.tensor_copy(smf[:], sm_u8[:])
        nsmf = const.tile([128, ST], F32)
        nc.vector.tensor_scalar(out=nsmf[:], in0=sm_u8[:], scalar1=-1.0,
                                scalar2=1.0, op0=AL.mult, op1=AL.add)
        xm_u8 = const.tile([1, LC], U8)
        nc.sync.dma_start(out=xm_u8[:], in_=xm.rearrange("(one l) -> one l",
                                                         one=1))
        nx_row = const.tile([1, LC], F32R)
        nc.vector.tensor_scalar(out=nx_row[:], in0=xm_u8[:], scalar1=-1.0,
                                scalar2=1.0, op0=AL.mult, op1=AL.add)

        # ---------- persistent activations ----------
        xT_f = persist.tile([128, 2, LC], F32)       # true x (final add)
        xr = persist.tile([128, 2, LC], F32R)        # x for mlp rhs
        QT = persist.tile([128, 2, LC], F32R)
        KT = persist.tile([128, 2, S], F32R)
        V1 = persist.tile([128, ST, H, DH + 1], F32R)
        V2 = persist.tile([128, ST, H, DH + 1], F32R)
        OT = persist.tile([128, 2, LC], F32R)
        nxb = persist.tile([DH + 1, LC], F32)
        ct_early = ctx.enter_context(tc.tile_pool(name="ct_early", bufs=1))
        cts_early = []
        for g in range(2):
            t = ct_early.tile([128, 2, LC], F32, name=f"cte_{g}", tag=f"cte{g}")
            nc.sync.dma_start(out=t[:], in_=CT[g * 256:(g + 1) * 256, :]
                              .rearrange("(j p) l -> p j l", p=128))
            cts_early.append(t)

        # ---------- phase 1: projections ----------
        with ExitStack() as c1:
            p1io = c1.enter_context(tc.tile_pool(name="p1io", bufs=1))
            p1ps = c1.enter_context(tc.tile_pool(name="p1ps", bufs=4,
                                                 space="PSUM"))
            xpe_f = p1io.tile([128, 2, LC], F32, tag="xio")
            nc.sync.dma_start(out=xT_f[:], in_=xT.rearrange(
                "(t p) l -> p t l", p=128))
            nc.sync.dma_start(out=xpe_f[:], in_=xpeT.rearrange(
                "(t p) l -> p t l", p=128))
            qin = p1io.tile([128, 2, LC], F32R, tag="qin")
            nc.vector.tensor_add(qin[:], xT_f[:], xpe_f[:])
            nc.vector.tensor_copy(xr[:], xT_f[:])

            # source loads in 512-col chunks; kin = src+srcpe, srcr = round(src)
            kin = p1io.tile([128, 2, S], F32R, tag="kin")
            srcr = p1io.tile([128, 2, S], F32R, tag="srcr")
            for nt in range(4):
                cs = slice(nt * 512, (nt + 1) * 512)
                sf = p1io.tile([128, 2, 512], F32, tag="sio", bufs=2,
                               name=f"sf_{nt}")
                pf = p1io.tile([128, 2, 512], F32, tag="pio", bufs=2,
                               name=f"pf_{nt}")
                nc.sync.dma_start(out=sf[:], in_=srcT[:, cs].rearrange(
                    "(t p) s -> p t s", p=128))
                nc.sync.dma_start(out=pf[:], in_=srcpeT[:, cs].rearrange(
                    "(t p) s -> p t s", p=128))
                nc.vector.tensor_add(kin[:, :, cs], sf[:], pf[:])
                nc.vector.tensor_copy(srcr[:, :, cs], sf[:])

            # Q^T [256, LC]
            for mt in range(2):
                ps = p1ps.tile([128, LC], F32, tag="qk", name=f"qps_{mt}")
                for kt in range(2):
                    nc.tensor.matmul(ps[:], wq_r[:, kt, mt * 128:(mt + 1) * 128],
                                     qin[:, kt, :], start=(kt == 0),
                                     stop=(kt == 1))
                nc.scalar.activation(QT[:, mt, :], ps[:], AF.Copy)
            # K^T [256, S]
            for mt in range(2):
                for nt in range(4):
                    ps = p1ps.tile([128, 512], F32, tag="qk",
                                   name=f"kps_{mt}_{nt}")
                    for kt in range(2):
                        nc.tensor.matmul(
                            ps[:], wk_r[:, kt, mt * 128:(mt + 1) * 128],
                            kin[:, kt, nt * 512:(nt + 1) * 512],
                            start=(kt == 0), stop=(kt == 1))
                    nc.scalar.activation(KT[:, mt, nt * 512:(nt + 1) * 512],
                                         ps[:], AF.Copy)
            # V [s, 256] -> vsb (ACT), V1 = V*sm (gpsimd TS), V2 = V - V1
            vsb = p1io.tile([128, ST, D], F32, tag="vsb")
            for i in range(ST):
                ps = p1ps.tile([128, D], F32, tag="vps", bufs=2,
                               name=f"vps_{i}")
                for kt in range(2):
                    nc.tensor.matmul(ps[:], srcr[:, kt, i * 128:(i + 1) * 128],
                                     wv_r[:, kt, :], start=(kt == 0),
                                     stop=(kt == 1))
                nc.scalar.activation(vsb[:, i, :], ps[:], AF.Copy)
            for i in range(ST):
                vsb_h = vsb[:, i, :].rearrange("p (h d) -> p h d", h=H)
                nc.vector.tensor_scalar(
                    out=V1[:, i, :, 0:DH], in0=vsb_h,
                    scalar1=smf[:, i:i + 1], scalar2=None, op0=AL.mult)
            nc.vector.tensor_sub(
                V2[:, :, :, 0:DH],
                vsb[:].rearrange("p i (h d) -> p i h d", h=H),
                V1[:, :, :, 0:DH])
            # sm / (1-sm) columns, broadcast over heads via 0-step AP
            sm_b = bass.AP(tensor=smf.tensor, offset=smf[:].offset,
                           ap=[smf[:].ap[0], smf[:].ap[1], [0, H], [0, 1]])
            nsm_b = bass.AP(tensor=nsmf.tensor, offset=nsmf[:].offset,
                            ap=[nsmf[:].ap[0], nsmf[:].ap[1], [0, H], [0, 1]])
            nc.vector.tensor_copy(V1[:, :, :, DH:DH + 1], sm_b)
            nc.vector.tensor_copy(V2[:, :, :, DH:DH + 1], nsm_b)

            # nxb [65, LC] via ones-matmul broadcast of nx_row
            psb = p1ps.tile([DH + 1, LC], F32, tag="nxbps", bufs=1)
            nc.tensor.matmul(psb[:], ones_r[0:1, 0:DH + 1], nx_row[:],
                             start=True, stop=True)
            nc.vector.tensor_copy(nxb[:], psb[:])

        # ---------- phase 2: attention ----------
        with ExitStack() as c2:
            sc_ps = c2.enter_context(tc.tile_pool(name="sc_ps", bufs=2,
                                                  space="PSUM"))
            av_ps = c2.enter_context(tc.tile_pool(name="av_ps", bufs=4,
                                                  space="PSUM"))
            p2sb = c2.enter_context(tc.tile_pool(name="p2sb", bufs=2))
            ct_pool = c2.enter_context(tc.tile_pool(name="ct", bufs=1))
            e_pool = c2.enter_context(tc.tile_pool(name="e", bufs=4))
            cts = list(cts_early)
            for g in range(2, 8):
                t = ct_pool.tile([128, 2, LC], F32, name=f"ct_{g}",
                                 tag=f"ct{g}")
                nc.sync.dma_start(out=t[:], in_=CT[g * 256:(g + 1) * 256, :]
                                  .rearrange("(j p) l -> p j l", p=128))
                cts.append(t)

            for hp in range(2):
                pv_ps = []
                for h2 in range(2):
                    pv_ps.append([av_ps.tile([DH + 1, LC], F32, tag="pv",
                                             name=f"pv_{hp}_{h2}_{v}")
                                  for v in range(2)])
                ehalf = {}
                for half in range(2):
                    for h2 in range(2):
                        ehalf[(half, h2)] = e_pool.tile(
                            [128, 8, LC], F32R, tag="eh",
                            name=f"eh_{hp}_{half}_{h2}")
                        ps_lo = h2 * DH
                        for g in range(4):
                            sc = sc_ps.tile([128, 2, LC], F32, tag="sc",
                                            name=f"sc_{hp}_{half}_{g}_{h2}")
                            for j in range(2):
                                i = half * 8 + g * 2 + j
                                nc.tensor.matmul(
                                    sc[:, j, :],
                                    KT[ps_lo:ps_lo + DH, hp,
                                       i * 128:(i + 1) * 128],
                                    QT[ps_lo:ps_lo + DH, hp, :],
                                    start=True, stop=True)
                            i0 = g * 2
                            nc.vector.tensor_mul(
                                ehalf[(half, h2)][:, i0:i0 + 2, :],
                                sc[:],
                                cts[half * 4 + g][:])
                    for h2 in range(2):
                        for ec in range(8):
                            eh_ap = ehalf[(half, h2)][:, ec:ec + 1, :] \
                                .rearrange("p i l -> p (i l)")
                            nc.scalar.activation(eh_ap, eh_ap.bitcast(F32),
                                                 AF.Exp)
                        h = 2 * hp + h2
                        for j in range(8):
                            i = half * 8 + j
                            for v, pv in enumerate(pv_ps[h2]):
                                vt = (V1 if v == 0 else V2)
                                nc.tensor.matmul(
                                    pv[:], vt[:, i, h, :],
                                    ehalf[(half, h2)][:, j, :],
                                    start=(i == 0), stop=(i == ST - 1),
                                    skip_group_check=True)
                # combines
                for h2 in range(2):
                    h = 2 * hp + h2
                    p1t, p2t = pv_ps[h2]
                    tmp = p2sb.tile([DH + 1, LC], F32, tag="tmp")
                    nc.vector.tensor_mul(tmp[:], nxb[:], p2t[:])
                    ocomb = p2sb.tile([DH + 1, LC], F32, tag="oc")
                    nc.vector.tensor_add(ocomb[:], tmp[:], p1t[:])
                    rr = p2sb.tile([128, LC], F32R, tag="rr")
                    nc.vector.reciprocal(rr[DH:DH + 1, :],
                                         ocomb[DH:DH + 1, :])
                    rb = av_ps.tile([DH, LC], F32, tag="pv")
                    nc.tensor.matmul(rb[:], ones_r[DH:DH + 1, 0:DH],
                                     rr[DH:DH + 1, :], start=True, stop=True)
                    otmp = p2sb.tile([DH, LC], F32R, tag="ot")
                    nc.vector.scalar_tensor_tensor(
                        out=otmp[:], in0=ocomb[0:DH, :], scalar=1.0,
                        in1=rb[:], op0=AL.mult, op1=AL.mult)
                    nc.sync.dma_start(out=OT[h2 * DH:(h2 + 1) * DH, hp, :],
                                      in_=otmp[:])

        # ---------- phase 3: merge + LN1 + MLP + LN2 + residual ----------
        with ExitStack() as c3:
            p3ps = c3.enter_context(tc.tile_pool(name="p3ps", bufs=2,
                                                 space="PSUM"))
            p3sb = c3.enter_context(tc.tile_pool(name="p3sb", bufs=2))

            def layernorm_T(msb, gcol, grow, brow, out_r, out_f32=None,
                            add_x=False):
                """msb: [128,2,LC] f32r sbuf (features on partitions).
                Writes normalized result to out_r (f32r) or, if add_x,
                writes out_f32 = LN(msb) + xT_f."""
                msq = p3sb.tile([128, 2, LC], F32R, tag="msq")
                nc.gpsimd.tensor_mul(msq[:], msb[:], msb[:])
                mean_b = p3ps.tile([128, LC], F32, tag="lnps", bufs=4)
                for kt in range(2):
                    nc.tensor.matmul(mean_b[:], i256_r[:], msb[:, kt, :],
                                     start=(kt == 0), stop=(kt == 1))
                ssq = p3ps.tile([1, LC], F32, tag="lnrow", bufs=1)
                for kt in range(2):
                    nc.tensor.matmul(ssq[:], ones_r[:, 0:1], msq[:, kt, :],
                                     start=(kt == 0), stop=(kt == 1))
                mrow = p3sb.tile([1, LC], F32, tag="lnr1")
                nc.vector.tensor_copy(mrow[:], mean_b[0:1, :])
                m2 = p3sb.tile([1, LC], F32, tag="lnr2")
                nc.vector.tensor_mul(m2[:], mrow[:], mrow[:])
                varr = p3sb.tile([1, LC], F32, tag="lnr3")
                nc.vector.scalar_tensor_tensor(
                    out=varr[:], in0=ssq[:], scalar=1.0 / D, in1=m2[:],
                    op0=AL.mult, op1=AL.subtract)
                sd = p3sb.tile([1, LC], F32, tag="lnr4")
                nc.scalar.activation(sd[:], varr[:], AF.Sqrt, bias=eps_t[:])
                rstd = p3sb.tile([1, LC], F32R, tag="lnr5")
                nc.vector.reciprocal(rstd[:], sd[:])
                mr = p3sb.tile([1, LC], F32R, tag="lnr6")
                nc.vector.tensor_mul(mr[:], mrow[:], rstd[:].bitcast(F32))
                ab = p3ps.tile([128, LC], F32, tag="lnps", bufs=4)
                nc.tensor.matmul(ab[:], ones_r[0:1, :], rstd[:],
                                 start=True, stop=True)
                for t in range(2):
                    bfull = p3ps.tile([128, LC], F32, tag="lnps", bufs=4)
                    nc.tensor.matmul(bfull[:],
                                     grow[:, t * 128:(t + 1) * 128], mr[:],
                                     start=True, stop=False)
                    nc.tensor.matmul(bfull[:],
                                     brow[:, t * 128:(t + 1) * 128],
                                     negones_row[:], start=False, stop=True)
                    tmp = p3sb.tile([128, LC], F32, tag="lnt")
                    nc.vector.scalar_tensor_tensor(
                        out=tmp[:], in0=msb[:, t, :], scalar=gcol[:, t:t + 1],
                        in1=ab[:], op0=AL.mult, op1=AL.mult)
                    if add_x:
                        u = p3sb.tile([128, LC], F32, tag="lnu")
                        nc.vector.tensor_sub(u[:], tmp[:], bfull[:])
                        nc.gpsimd.tensor_add(out_f32[:, t, :], u[:],
                                             xT_f[:, t, :])
                    else:
                        nc.vector.tensor_sub(out_r[:, t, :], tmp[:],
                                             bfull[:])

            # merge: M^T = Wm^T @ OT
            msb = p3sb.tile([128, 2, LC], F32R, tag="msb")
            for mt in range(2):
                ps = p3ps.tile([128, LC], F32, tag="mm")
                for kt in range(2):
                    nc.tensor.matmul(ps[:], wm_r[:, kt, mt * 128:(mt + 1) * 128],
                                     OT[:, kt, :], start=(kt == 0),
                                     stop=(kt == 1))
                nc.scalar.activation(msb[:, mt, :], ps[:], AF.Copy)
            msg = p3sb.tile([128, 2, LC], F32R, tag="msg")
            layernorm_T(msb, g1col, g1row, b1row, msg)

            # mlp1 + relu
            relu = p3sb.tile([128, 4, LC], F32R, tag="relu")
            for ft in range(4):
                ps = p3ps.tile([128, LC], F32, tag="mm")
                for kt in range(4):
                    rhs = xr[:, kt, :] if kt < 2 else msg[:, kt - 2, :]
                    nc.tensor.matmul(ps[:],
                                     w1_r[:, kt, ft * 128:(ft + 1) * 128],
                                     rhs, start=(kt == 0), stop=(kt == 3))
                nc.scalar.activation(relu[:, ft, :], ps[:], AF.Relu)
            # mlp2
            m2sb = p3sb.tile([128, 2, LC], F32R, tag="m2sb")
            for mt in range(2):
                ps = p3ps.tile([128, LC], F32, tag="mm")
                for kt in range(4):
                    nc.tensor.matmul(ps[:],
                                     w2_r[:, kt, mt * 128:(mt + 1) * 128],
                                     relu[:, kt, :], start=(kt == 0),
                                     stop=(kt == 3))
                nc.scalar.activation(m2sb[:, mt, :], ps[:], AF.Copy)
            final = p3sb.tile([128, 2, LC], F32, tag="final")
            layernorm_T(m2sb, g2col, g2row, b2row, None, out_f32=final,
                        add_x=True)
            nc.sync.dma_start(out=outT.rearrange("(t p) l -> p t l", p=128),
                              in_=final[:])

    nc.compile()
    return nc


def _get_nc():
    global _CACHED_NC
    if _CACHED_NC is None:
        _CACHED_NC = build_nc()
    return _CACHED_NC


def kernel(x, source, x_pe, source_pe, x_mask, source_mask, compatibility,
           Wq, Wk, Wv, Wmerge, Wmlp1, Wmlp2, ln1_g, ln1_b, ln2_g, ln2_b):
    nc = _get_nc()
    f32 = np.float32
    xT = np.ascontiguousarray(np.asarray(x, f32).transpose(0, 2, 1))
    xpeT = np.ascontiguousarray(np.asarray(x_pe, f32).transpose(0, 2, 1))
    srcT = np.ascontiguousarray(np.asarray(source, f32).transpose(0, 2, 1))
    srcpeT = np.ascontiguousarray(
        np.asarray(source_pe, f32).transpose(0, 2, 1))
    CT = np.ascontiguousarray(
        np.asarray(compatibility, f32).transpose(0, 2, 1))
    xmu = np.asarray(x_mask).astype(np.uint8)
    smu = np.asarray(source_mask).astype(np.uint8)
    weights = {
        "Wq": np.asarray(Wq, f32), "Wk": np.asarray(Wk, f32),
        "Wv": np.asarray(Wv, f32), "Wm": np.asarray(Wmerge, f32),
        "W1": np.asarray(Wmlp1, f32), "W2": np.asarray(Wmlp2, f32),
        "g1": np.asarray(ln1_g, f32), "b1": np.asarray(ln1_b, f32),
        "g2": np.asarray(ln2_g, f32), "b2": np.asarray(ln2_b, f32),
    }
    in_maps = []
    for c in range(NCORES):
        b, l0 = c // (NCORES // BS), (c % (NCORES // BS)) * LC
        sl = slice(l0, l0 + LC)
        in_maps.append({
            "xT": np.ascontiguousarray(xT[b][:, sl]),
            "xpeT": np.ascontiguousarray(xpeT[b][:, sl]),
            "srcT": srcT[b],
            "srcpeT": srcpeT[b],
            "CT": np.ascontiguousarray(CT[b][:, sl]),
            "xm": np.ascontiguousarray(xmu[b][sl]),
            "sm": smu[b],
            **weights,
        })
    res = run_bass_kernel_spmd(nc, in_maps, list(range(NCORES)))
    outT = np.empty((BS, D, L), f32)
    for c in range(NCORES):
        b, l0 = c // (NCORES // BS), (c % (NCORES // BS)) * LC
        outT[b][:, l0:l0 + LC] = res.results[c]["outT"]
    return np.ascontiguousarray(outT.transpose(0, 2, 1))


# revision 35
# speedup vs baseline: 1.0143x; 1.0110x over previous
"""CorrespondenceAttentionLayer on 8 TRN2 NeuronCores (Bass/Tile).

Strategy: row-shard the bs*L=4096 query rows into 8 shards of 512 (4 cores
per batch). No collectives; source-side K/V recomputed per core. Scores are
computed transposed (S^T[s,l]) so softmax normalization reduces over the
PSUM/matmul axis; masking is handled by a double AV matmul against
V*source_mask and V*(1-source_mask) with a ones-augmented column carrying
the softmax denominators. Matmuls run in float32r (1 cyc/row on PE).

kernel(**inputs) takes full unsharded inputs and returns the full output.
Host work is layout-only (transposes/slices).
"""
import os
import sys
from contextlib import ExitStack

import numpy as np

for _p in ("/opt/trn_rl_repo", "/root/.axon_site/_ro/trn_rl_repo"):
    if os.path.isdir(_p) and _p not in sys.path:
        sys.path.append(_p)

import concourse.bass as bass  # noqa: E402
import concourse.tile as tile  # noqa: E402
from concourse import bacc, mybir  # noqa: E402
from concourse.bass_utils import run_bass_kernel_spmd  # noqa: E402

BS, L, S, D = 2, 2048, 2048, 256
H, DH = 4, 64
NCORES = 8
LC = BS * L // NCORES      # 512 query rows per core
ST = S // 128              # 16 source tiles
F32 = mybir.dt.float32
F32R = mybir.dt.float32r
U8 = mybir.dt.uint8
AL = mybir.AluOpType
AF = mybir.ActivationFunctionType

_CACHED_NC = None


def build_nc():
    nc = bacc.Bacc("TRN2", target_bir_lowering=False, debug=False,
                   num_devices=NCORES)

    def din(name, shape, dt=F32):
        return nc.dram_tensor(name, shape, dt, kind="ExternalInput").ap()

    xT = din("xT", [D, LC])
    xpeT = din("xpeT", [D, LC])
    srcT = din("srcT", [D, S])
    srcpeT = din("srcpeT", [D, S])
    CT = din("CT", [S, LC])
    xm = din("xm", [LC], U8)
    sm = din("sm", [S], U8)
    Wq = din("Wq", [D, D])
    Wk = din("Wk", [D, D])
    Wv = din("Wv", [D, D])
    Wm = din("Wm", [D, D])
    W1 = din("W1", [2 * D, 2 * D])
    W2 = din("W2", [2 * D, D])
    g1 = din("g1", [D])
    b1 = din("b1", [D])
    g2 = din("g2", [D])
    b2 = din("b2", [D])
    outT = nc.dram_tensor("outT", [D, LC], F32, kind="ExternalOutput").ap()

    with tile.TileContext(nc) as tc, ExitStack() as ctx, \
            nc.allow_low_precision(reason="float32r matmul pipeline"):
        const = ctx.enter_context(tc.tile_pool(name="const", bufs=1))
        persist = ctx.enter_context(tc.tile_pool(name="persist", bufs=1))

        # ---------- constants ----------
        ones_f = const.tile([128, 128], F32)
        nc.vector.memset(ones_f[:], 1.0)
        ones_r = const.tile([128, 128], F32R)
        nc.vector.tensor_copy(ones_r[:], ones_f[:])
        i256_r = const.tile([128, 128], F32R)  # 1/256 everywhere
        nc.vector.tensor_scalar(out=i256_r[:], in0=ones_f[:], scalar1=1.0 / D,
                                scalar2=None, op0=AL.mult)
        negones_row = const.tile([1, LC], F32R)
        nc.vector.tensor_scalar(out=negones_row[:], in0=ones_f[0:1, 0:1],
                                scalar1=-1.0, scalar2=None, op0=AL.mult) \
            if False else None
        # build -1 row via memset+copy (memset on f32, copy to f32r)
        negones_f = const.tile([1, LC], F32)
        nc.vector.memset(negones_f[:], -1.0)
        nc.vector.tensor_copy(negones_row[:], negones_f[:])
        eps_t = const.tile([1, 1], F32)
        nc.vector.memset(eps_t[:], 1e-5)

        # ---------- weights (f32r, 1/8 folded into Wq) ----------
        wload = ctx.enter_context(tc.tile_pool(name="wload", bufs=1))

        def load_w(ap, kt, cols, scale=None):
            wf = wload.tile([128, kt, cols], F32, tag="wf")
            nc.sync.dma_start(out=wf[:], in_=ap.rearrange(
                "(t p) m -> p t m", p=128))
            wr = const.tile([128, kt, cols], F32R)
            if scale is None:
                nc.gpsimd.tensor_copy(wr[:], wf[:])
            else:
                nc.gpsimd.tensor_scalar(out=wr[:], in0=wf[:], scalar1=scale,
                                        scalar2=None, op0=AL.mult)
            return wr

        wq_r = load_w(Wq, 2, D, scale=1.0 / np.sqrt(DH))
        wk_r = load_w(Wk, 2, D)
        wv_r = load_w(Wv, 2, D)
        wm_r = load_w(Wm, 2, D)
        w1_r = load_w(W1, 4, 2 * D)
        w2_r = load_w(W2, 4, D)

        # LN params: per-partition cols [128,2] and f32r rows [1,256]
        def load_ln(gap, bap):
            gcol = const.tile([128, 2], F32)
            nc.sync.dma_start(out=gcol[:], in_=gap.rearrange("(t p) -> p t",
                                                             p=128))
            grow_f = const.tile([1, D], F32)
            nc.sync.dma_start(out=grow_f[:], in_=gap.rearrange(
                "(one d) -> one d", one=1))
            grow = const.tile([1, D], F32R)
            nc.vector.tensor_copy(grow[:], grow_f[:])
            brow_f = const.tile([1, D], F32)
            nc.sync.dma_start(out=brow_f[:], in_=bap.rearrange(
                "(one d) -> one d", one=1))
            brow = const.tile([1, D], F32R)
            nc.vector.tensor_copy(brow[:], brow_f[:])
            return gcol, grow, brow

        g1col, g1row, b1row = load_ln(g1, b1)
        g2col, g2row, b2row = load_ln(g2, b2)

        # ---------- masks ----------
        sm_u8 = const.tile([128, ST], U8)
        nc.sync.dma_start(out=sm_u8[:], in_=sm.rearrange("(i p) -> p i",
                                                         p=128))
        smf = const.tile([128, ST], F32)
        nc.vector.tensor_copy(smf[:], sm_u8[:])
        nsmf = const.tile([128, ST], F32)
        nc.vector.tensor_scalar(out=nsmf[:], in0=sm_u8[:], scalar1=-1.0,
                                scalar2=1.0, op0=AL.mult, op1=AL.add)
        xm_u8 = const.tile([1, LC], U8)
        nc.sync.dma_start(out=xm_u8[:], in_=xm.rearrange("(one l) -> one l",
                                                         one=1))
        nx_row = const.tile([1, LC], F32R)
        nc.vector.tensor_scalar(out=nx_row[:], in0=xm_u8[:], scalar1=-1.0,
                                scalar2=1.0, op0=AL.mult, op1=AL.add)

        # ---------- persistent activations ----------
        xT_f = persist.tile([128, 2, LC], F32)       # true x (final add)
        xr = persist.tile([128, 2, LC], F32R)        # x for mlp rhs
        QT = persist.tile([128, 2, LC], F32R)
        KT = persist.tile([128, 2, S], F32R)
        V1 = persist.tile([128, ST, H, DH + 1], F32R)
        V2 = persist.tile([128, ST, H, DH + 1], F32R)
        OT = persist.tile([128, 2, LC], F32R)
        nxb = persist.tile([DH + 1, LC], F32)
        ct_early = ctx.enter_context(tc.tile_pool(name="ct_early", bufs=1))
        cts_early = []
        for g in range(2):
            t = ct_early.tile([128, 2, LC], F32, name=f"cte_{g}", tag=f"cte{g}")
            nc.sync.dma_start(out=t[:], in_=CT[g * 256:(g + 1) * 256, :]
                              .rearrange("(j p) l -> p j l", p=128))
            cts_early.append(t)

        # ---------- phase 1: projections ----------
        with ExitStack() as c1:
            p1io = c1.enter_context(tc.tile_pool(name="p1io", bufs=1))
            p1ps = c1.enter_context(tc.tile_pool(name="p1ps", bufs=4,
                                                 space="PSUM"))
            xpe_f = p1io.tile([128, 2, LC], F32, tag="xio")
            nc.sync.dma_start(out=xT_f[:], in_=xT.rearrange(
                "(t p) l -> p t l", p=128))
            nc.sync.dma_start(out=xpe_f[:], in_=xpeT.rearrange(
                "(t p) l -> p t l", p=128))
            qin = p1io.tile([128, 2, LC], F32R, tag="qin")
            nc.vector.tensor_add(qin[:], xT_f[:], xpe_f[:])
            nc.vector.tensor_copy(xr[:], xT_f[:])

            # source loads in 512-col chunks; kin = src+srcpe, srcr = round(src)
            kin = p1io.tile([128, 2, S], F32R, tag="kin")
            srcr = p1io.tile([128, 2, S], F32R, tag="srcr")
            for nt in range(4):
                cs = slice(nt * 512, (nt + 1) * 512)
                sf = p1io.tile([128, 2, 512], F32, tag="sio", bufs=2,
                               name=f"sf_{nt}")
                pf = p1io.tile([128, 2, 512], F32, tag="pio", bufs=2,
                               name=f"pf_{nt}")
                nc.sync.dma_start(out=sf[:], in_=srcT[:, cs].rearrange(
                    "(t p) s -> p t s", p=128))
                nc.sync.dma_start(out=pf[:], in_=srcpeT[:, cs].rearrange(
                    "(t p) s -> p t s", p=128))
                nc.vector.tensor_add(kin[:, :, cs], sf[:], pf[:])
                nc.vector.tensor_copy(srcr[:, :, cs], sf[:])

            # Q^T [256, LC]: both mt tiles into one 2-bank psum, one copy
            qps = p1ps.tile([128, 2, LC], F32, tag="qk", bufs=2, name="qps")
            for mt in range(2):
                for kt in range(2):
                    nc.tensor.matmul(qps[:, mt, :],
                                     wq_r[:, kt, mt * 128:(mt + 1) * 128],
                                     qin[:, kt, :], start=(kt == 0),
                                     stop=(kt == 1))
            nc.scalar.activation(QT[:].rearrange("p t l -> p (t l)"),
                                 qps[:].rearrange("p t l -> p (t l)"),
                                 AF.Copy)
            # K^T [256, S]: nt pairs share a 2-bank psum, one copy per pair
            for mt in range(2):
                for np2 in range(2):
                    ps = p1ps.tile([128, 2, 512], F32, tag="qk", bufs=2,
                                   name=f"kps_{mt}_{np2}")
                    for j in range(2):
                        nt = np2 * 2 + j
                        for kt in range(2):
                            nc.tensor.matmul(
                                ps[:, j, :],
                                wk_r[:, kt, mt * 128:(mt + 1) * 128],
                                kin[:, kt, nt * 512:(nt + 1) * 512],
                                start=(kt == 0), stop=(kt == 1))
                    nc.scalar.activation(
                        KT[:, mt, np2 * 1024:(np2 + 1) * 1024],
                        ps[:].rearrange("p j l -> p (j l)"), AF.Copy)
            # V [s, 256] -> vsb (ACT), V1 = V*sm (gpsimd TS), V2 = V - V1
            vsb = p1io.tile([128, ST, D], F32, tag="vsb")
            for i in range(ST):
                ps = p1ps.tile([128, D], F32, tag="vps", bufs=2,
                               name=f"vps_{i}")
                for kt in range(2):
                    nc.tensor.matmul(ps[:], srcr[:, kt, i * 128:(i + 1) * 128],
                                     wv_r[:, kt, :], start=(kt == 0),
                                     stop=(kt == 1))
                nc.scalar.activation(vsb[:, i, :], ps[:], AF.Copy)
            for i in range(ST):
                vsb_h = vsb[:, i, :].rearrange("p (h d) -> p h d", h=H)
                nc.vector.tensor_scalar(
                    out=V1[:, i, :, 0:DH], in0=vsb_h,
                    scalar1=smf[:, i:i + 1], scalar2=None, op0=AL.mult)
            nc.vector.tensor_sub(
                V2[:, :, :, 0:DH],
                vsb[:].rearrange("p i (h d) -> p i h d", h=H),
                V1[:, :, :, 0:DH])
            # sm / (1-sm) columns, broadcast over heads via 0-step AP
            sm_b = bass.AP(tensor=smf.tensor, offset=smf[:].offset,
                           ap=[smf[:].ap[0], smf[:].ap[1], [0, H], [0, 1]])
            nsm_b = bass.AP(tensor=nsmf.tensor, offset=nsmf[:].offset,
                            ap=[nsmf[:].ap[0], nsmf[:].ap[1], [0, H], [0, 1]])
            nc.vector.tensor_copy(V1[:, :, :, DH:DH + 1], sm_b)
            nc.vector.tensor_copy(V2[:, :, :, DH:DH + 1], nsm_b)

            # nxb [65, LC] via ones-matmul broadcast of nx_row
            psb = p1ps.tile([DH + 1, LC], F32, tag="nxbps", bufs=1)
            nc.tensor.matmul(psb[:], ones_r[0:1, 0:DH + 1], nx_row[:],
                             start=True, stop=True)
            nc.vector.tensor_copy(nxb[:], psb[:])

        # ---------- phase 2: attention ----------
        with ExitStack() as c2:
            sc_ps = c2.enter_context(tc.tile_pool(name="sc_ps", bufs=2,
                                                  space="PSUM"))
            av_ps = c2.enter_context(tc.tile_pool(name="av_ps", bufs=4,
                                                  space="PSUM"))
            p2sb = c2.enter_context(tc.tile_pool(name="p2sb", bufs=2))
            ct_pool = c2.enter_context(tc.tile_pool(name="ct", bufs=1))
            e_pool = c2.enter_context(tc.tile_pool(name="e", bufs=4))
            cts = list(cts_early)
            for g in range(2, 8):
                t = ct_pool.tile([128, 2, LC], F32, name=f"ct_{g}",
                                 tag=f"ct{g}")
                nc.sync.dma_start(out=t[:], in_=CT[g * 256:(g + 1) * 256, :]
                                  .rearrange("(j p) l -> p j l", p=128))
                cts.append(t)

            for hp in range(2):
                pv_ps = []
                for h2 in range(2):
                    pv_ps.append([av_ps.tile([DH + 1, LC], F32, tag="pv",
                                             name=f"pv_{hp}_{h2}_{v}")
                                  for v in range(2)])
                ehalf = {}
                for half in range(2):
                    for h2 in range(2):
                        ehalf[(half, h2)] = e_pool.tile(
                            [128, 8, LC], F32R, tag="eh",
                            name=f"eh_{hp}_{half}_{h2}")
                        ps_lo = h2 * DH
                        for g in range(4):
                            sc = sc_ps.tile([128, 2, LC], F32, tag="sc",
                                            name=f"sc_{hp}_{half}_{g}_{h2}")
                            for j in range(2):
                                i = half * 8 + g * 2 + j
                                nc.tensor.matmul(
                                    sc[:, j, :],
                                    KT[ps_lo:ps_lo + DH, hp,
                                       i * 128:(i + 1) * 128],
                                    QT[ps_lo:ps_lo + DH, hp, :],
                                    start=True, stop=True)
                            i0 = g * 2
                            nc.vector.tensor_mul(
                                ehalf[(half, h2)][:, i0:i0 + 2, :],
                                sc[:],
                                cts[half * 4 + g][:])
                    for h2 in range(2):
                        for ec in range(8):
                            eh_ap = ehalf[(half, h2)][:, ec:ec + 1, :] \
                                .rearrange("p i l -> p (i l)")
                            nc.scalar.activation(eh_ap, eh_ap.bitcast(F32),
                                                 AF.Exp)
                        h = 2 * hp + h2
                        for j in range(8):
                            i = half * 8 + j
                            for v, pv in enumerate(pv_ps[h2]):
                                vt = (V1 if v == 0 else V2)
                                nc.tensor.matmul(
                                    pv[:], vt[:, i, h, :],
                                    ehalf[(half, h2)][:, j, :],
                                    start=(i == 0), stop=(i == ST - 1),
                                    skip_group_check=True)
                # combines
                for h2 in range(2):
                    h = 2 * hp + h2
                    p1t, p2t = pv_ps[h2]
                    tmp = p2sb.tile([DH + 1, LC], F32, tag="tmp")
                    nc.vector.tensor_mul(tmp[:], nxb[:], p2t[:])
                    ocomb = p2sb.tile([DH + 1, LC], F32, tag="oc")
                    nc.vector.tensor_add(ocomb[:], tmp[:], p1t[:])
                    rr = p2sb.tile([128, LC], F32R, tag="rr")
                    nc.vector.reciprocal(rr[DH:DH + 1, :],
                                         ocomb[DH:DH + 1, :])
                    rb = av_ps.tile([DH, LC], F32, tag="pv")
                    nc.tensor.matmul(rb[:], ones_r[DH:DH + 1, 0:DH],
                                     rr[DH:DH + 1, :], start=True, stop=True)
                    otmp = p2sb.tile([DH, LC], F32R, tag="ot")
                    nc.vector.scalar_tensor_tensor(
                        out=otmp[:], in0=ocomb[0:DH, :], scalar=1.0,
                        in1=rb[:], op0=AL.mult, op1=AL.mult)
                    nc.sync.dma_start(out=OT[h2 * DH:(h2 + 1) * DH, hp, :],
                                      in_=otmp[:])

        # ---------- phase 3: merge + LN1 + MLP + LN2 + residual ----------
        with ExitStack() as c3:
            p3ps = c3.enter_context(tc.tile_pool(name="p3ps", bufs=2,
                                                 space="PSUM"))
            p3sb = c3.enter_context(tc.tile_pool(name="p3sb", bufs=2))

            def layernorm_T(msb, gcol, grow, brow, out_r, out_f32=None,
                            add_x=False):
                """msb: [128,2,LC] f32r sbuf (features on partitions).
                Writes normalized result to out_r (f32r) or, if add_x,
                writes out_f32 = LN(msb) + xT_f."""
                msq = p3sb.tile([128, 2, LC], F32R, tag="msq")
                nc.gpsimd.tensor_mul(msq[:], msb[:], msb[:])
                mean_b = p3ps.tile([128, LC], F32, tag="lnps", bufs=4)
                for kt in range(2):
                    nc.tensor.matmul(mean_b[:], i256_r[:], msb[:, kt, :],
                                     start=(kt == 0), stop=(kt == 1))
                ssq = p3ps.tile([1, LC], F32, tag="lnrow", bufs=1)
                for kt in range(2):
                    nc.tensor.matmul(ssq[:], ones_r[:, 0:1], msq[:, kt, :],
                                     start=(kt == 0), stop=(kt == 1))
                mrow = p3sb.tile([1, LC], F32, tag="lnr1")
                nc.vector.tensor_copy(mrow[:], mean_b[0:1, :])
                m2 = p3sb.tile([1, LC], F32, tag="lnr2")
                nc.vector.tensor_mul(m2[:], mrow[:], mrow[:])
                varr = p3sb.tile([1, LC], F32, tag="lnr3")
                nc.vector.scalar_tensor_tensor(
                    out=varr[:], in0=ssq[:], scalar=1.0 / D, in1=m2[:],
                    op0=AL.mult, op1=AL.subtract)
                sd = p3sb.tile([1, LC], F32, tag="lnr4")
                nc.scalar.activation(sd[:], varr[:], AF.Sqrt, bias=eps_t[:])
                rstd = p3sb.tile([1, LC], F32R, tag="lnr5")
                nc.vector.reciprocal(rstd[:], sd[:])
                mr = p3sb.tile([1, LC], F32R, tag="lnr6")
                nc.vector.tensor_mul(mr[:], mrow[:], rstd[:].bitcast(F32))
                ab = p3ps.tile([128, LC], F32, tag="lnps", bufs=4)
                nc.tensor.matmul(ab[:], ones_r[0:1, :], rstd[:],
                                 start=True, stop=True)
                for t in range(2):
                    bfull = p3ps.tile([128, LC], F32, tag="lnps", bufs=4)
                    nc.tensor.matmul(bfull[:],
                                     grow[:, t * 128:(t + 1) * 128], mr[:],
                                     start=True, stop=False)
                    nc.tensor.matmul(bfull[:],
                                     brow[:, t * 128:(t + 1) * 128],
                                     negones_row[:], start=False, stop=True)
                    tmp = p3sb.tile([128, LC], F32, tag="lnt")
                    nc.vector.scalar_tensor_tensor(
                        out=tmp[:], in0=msb[:, t, :], scalar=gcol[:, t:t + 1],
                        in1=ab[:], op0=AL.mult, op1=AL.mult)
                    if add_x:
                        u = p3sb.tile([128, LC], F32, tag="lnu")
                        nc.vector.tensor_sub(u[:], tmp[:], bfull[:])
                        nc.gpsimd.tensor_add(out_f32[:, t, :], u[:],
                                             xT_f[:, t, :])
                    else:
                        nc.vector.tensor_sub(out_r[:, t, :], tmp[:],
                                             bfull[:])

            # merge: M^T = Wm^T @ OT
            msb = p3sb.tile([128, 2, LC], F32R, tag="msb")
            for mt in range(2):
                ps = p3ps.tile([128, LC], F32, tag="mm")
                for kt in range(2):
                    nc.tensor.matmul(ps[:], wm_r[:, kt, mt * 128:(mt + 1) * 128],
                                     OT[:, kt, :], start=(kt == 0),
                                     stop=(kt == 1))
                nc.scalar.activation(msb[:, mt, :], ps[:], AF.Copy)
            msg = p3sb.tile([128, 2, LC], F32R, tag="msg")
            layernorm_T(msb, g1col, g1row, b1row, msg)

            # mlp1 + relu
            relu = p3sb.tile([128, 4, LC], F32R, tag="relu")
            for ft in range(4):
                ps = p3ps.tile([128, LC], F32, tag="mm")
                for kt in range(4):
                    rhs = xr[:, kt, :] if kt < 2 else msg[:, kt - 2, :]
                    nc.tensor.matmul(ps[:],
                                     w1_r[:, kt, ft * 128:(ft + 1) * 128],
                                     rhs, start=(kt == 0), stop=(kt == 3))
                nc.scalar.activation(relu[:, ft, :], ps[:], AF.Relu)
            # mlp2
            m2sb = p3sb.tile([128, 2, LC], F32R, tag="m2sb")
            for mt in range(2):
                ps = p3ps.tile([128, LC], F32, tag="mm")
                for kt in range(4):
                    nc.tensor.matmul(ps[:],
                                     w2_r[:, kt, mt * 128:(mt + 1) * 128],
                                     relu[:, kt, :], start=(kt == 0),
                                     stop=(kt == 3))
                nc.scalar.activation(m2sb[:, mt, :], ps[:], AF.Copy)
            final = p3sb.tile([128, 2, LC], F32, tag="final")
            layernorm_T(m2sb, g2col, g2row, b2row, None, out_f32=final,
                        add_x=True)
            nc.sync.dma_start(out=outT.rearrange("(t p) l -> p t l", p=128),
                              in_=final[:])

    nc.compile()
    return nc


def _get_nc():
    global _CACHED_NC
    if _CACHED_NC is None:
        _CACHED_NC = build_nc()
    return _CACHED_NC


def kernel(x, source, x_pe, source_pe, x_mask, source_mask, compatibility,
           Wq, Wk, Wv, Wmerge, Wmlp1, Wmlp2, ln1_g, ln1_b, ln2_g, ln2_b):
    nc = _get_nc()
    f32 = np.float32
    xT = np.ascontiguousarray(np.asarray(x, f32).transpose(0, 2, 1))
    xpeT = np.ascontiguousarray(np.asarray(x_pe, f32).transpose(0, 2, 1))
    srcT = np.ascontiguousarray(np.asarray(source, f32).transpose(0, 2, 1))
    srcpeT = np.ascontiguousarray(
        np.asarray(source_pe, f32).transpose(0, 2, 1))
    CT = np.ascontiguousarray(
        np.asarray(compatibility, f32).transpose(0, 2, 1))
    xmu = np.asarray(x_mask).astype(np.uint8)
    smu = np.asarray(source_mask).astype(np.uint8)
    weights = {
        "Wq": np.asarray(Wq, f32), "Wk": np.asarray(Wk, f32),
        "Wv": np.asarray(Wv, f32), "Wm": np.asarray(Wmerge, f32),
        "W1": np.asarray(Wmlp1, f32), "W2": np.asarray(Wmlp2, f32),
        "g1": np.asarray(ln1_g, f32), "b1": np.asarray(ln1_b, f32),
        "g2": np.asarray(ln2_g, f32), "b2": np.asarray(ln2_b, f32),
    }
    in_maps = []
    for c in range(NCORES):
        b, l0 = c // (NCORES // BS), (c % (NCORES // BS)) * LC
        sl = slice(l0, l0 + LC)
        in_maps.append({
            "xT": np.ascontiguousarray(xT[b][:, sl]),
            "xpeT": np.ascontiguousarray(xpeT[b][:, sl]),
            "srcT": srcT[b],
            "srcpeT": srcpeT[b],
            "CT": np.ascontiguousarray(CT[b][:, sl]),
            "xm": np.ascontiguousarray(xmu[b][sl]),
            "sm": smu[b],
            **weights,
        })
    res = run_bass_kernel_spmd(nc, in_maps, list(range(NCORES)))
    outT = np.empty((BS, D, L), f32)
    for c in range(NCORES):
        b, l0 = c // (NCORES // BS), (c % (NCORES // BS)) * LC
        outT[b][:, l0:l0 + LC] = res.results[c]["outT"]
    return np.ascontiguousarray(outT.transpose(0, 2, 1))


# revision 36
# speedup vs baseline: 1.0271x; 1.0126x over previous
"""CorrespondenceAttentionLayer on 8 TRN2 NeuronCores (Bass/Tile).

Strategy: row-shard the bs*L=4096 query rows into 8 shards of 512 (4 cores
per batch). No collectives; source-side K/V recomputed per core. Scores are
computed transposed (S^T[s,l]) so softmax normalization reduces over the
PSUM/matmul axis; masking is handled by a double AV matmul against
V*source_mask and V*(1-source_mask) with a ones-augmented column carrying
the softmax denominators. Matmuls run in float32r (1 cyc/row on PE).

kernel(**inputs) takes full unsharded inputs and returns the full output.
Host work is layout-only (transposes/slices).
"""
import os
import sys
from contextlib import ExitStack

import numpy as np

for _p in ("/opt/trn_rl_repo", "/root/.axon_site/_ro/trn_rl_repo"):
    if os.path.isdir(_p) and _p not in sys.path:
        sys.path.append(_p)

import concourse.bass as bass  # noqa: E402
import concourse.tile as tile  # noqa: E402
from concourse import bacc, mybir  # noqa: E402
from concourse.bass_utils import run_bass_kernel_spmd  # noqa: E402

BS, L, S, D = 2, 2048, 2048, 256
H, DH = 4, 64
NCORES = 8
LC = BS * L // NCORES      # 512 query rows per core
ST = S // 128              # 16 source tiles
F32 = mybir.dt.float32
F32R = mybir.dt.float32r
U8 = mybir.dt.uint8
AL = mybir.AluOpType
AF = mybir.ActivationFunctionType

_CACHED_NC = None


def build_nc():
    nc = bacc.Bacc("TRN2", target_bir_lowering=False, debug=False,
                   num_devices=NCORES)

    def din(name, shape, dt=F32):
        return nc.dram_tensor(name, shape, dt, kind="ExternalInput").ap()

    xT = din("xT", [D, LC])
    xpeT = din("xpeT", [D, LC])
    srcT = din("srcT", [D, S])
    srcpeT = din("srcpeT", [D, S])
    CT = din("CT", [S, LC])
    xm = din("xm", [LC], U8)
    sm = din("sm", [S], U8)
    Wq = din("Wq", [D, D])
    Wk = din("Wk", [D, D])
    Wv = din("Wv", [D, D])
    Wm = din("Wm", [D, D])
    W1 = din("W1", [2 * D, 2 * D])
    W2 = din("W2", [2 * D, D])
    g1 = din("g1", [D])
    b1 = din("b1", [D])
    g2 = din("g2", [D])
    b2 = din("b2", [D])
    outT = nc.dram_tensor("outT", [D, LC], F32, kind="ExternalOutput").ap()

    with tile.TileContext(nc) as tc, ExitStack() as ctx, \
            nc.allow_low_precision(reason="float32r matmul pipeline"):
        const = ctx.enter_context(tc.tile_pool(name="const", bufs=1))
        persist = ctx.enter_context(tc.tile_pool(name="persist", bufs=1))

        # ---------- constants ----------
        ones_f = const.tile([128, 128], F32)
        nc.vector.memset(ones_f[:], 1.0)
        ones_r = const.tile([128, 128], F32R)
        nc.vector.tensor_copy(ones_r[:], ones_f[:])
        i256_r = const.tile([128, 128], F32R)  # 1/256 everywhere
        nc.vector.tensor_scalar(out=i256_r[:], in0=ones_f[:], scalar1=1.0 / D,
                                scalar2=None, op0=AL.mult)
        negones_row = const.tile([1, LC], F32R)
        nc.vector.tensor_scalar(out=negones_row[:], in0=ones_f[0:1, 0:1],
                                scalar1=-1.0, scalar2=None, op0=AL.mult) \
            if False else None
        # build -1 row via memset+copy (memset on f32, copy to f32r)
        negones_f = const.tile([1, LC], F32)
        nc.vector.memset(negones_f[:], -1.0)
        nc.vector.tensor_copy(negones_row[:], negones_f[:])
        eps_t = const.tile([1, 1], F32)
        nc.vector.memset(eps_t[:], 1e-5)

        # ---------- weights (f32r, 1/8 folded into Wq) ----------
        wload = ctx.enter_context(tc.tile_pool(name="wload", bufs=1))

        def load_w(ap, kt, cols, scale=None):
            wf = wload.tile([128, kt, cols], F32, tag="wf")
            nc.sync.dma_start(out=wf[:], in_=ap.rearrange(
                "(t p) m -> p t m", p=128))
            wr = const.tile([128, kt, cols], F32R)
            if scale is None:
                nc.gpsimd.tensor_copy(wr[:], wf[:])
            else:
                nc.gpsimd.tensor_scalar(out=wr[:], in0=wf[:], scalar1=scale,
                                        scalar2=None, op0=AL.mult)
            return wr

        wq_r = load_w(Wq, 2, D, scale=1.0 / np.sqrt(DH))
        wk_r = load_w(Wk, 2, D)
        wv_r = load_w(Wv, 2, D)
        wm_r = load_w(Wm, 2, D)
        w1_r = load_w(W1, 4, 2 * D)
        w2_r = load_w(W2, 4, D)

        # LN params: per-partition cols [128,2] and f32r rows [1,256]
        def load_ln(gap, bap):
            gcol = const.tile([128, 2], F32)
            nc.sync.dma_start(out=gcol[:], in_=gap.rearrange("(t p) -> p t",
                                                             p=128))
            grow_f = const.tile([1, D], F32)
            nc.sync.dma_start(out=grow_f[:], in_=gap.rearrange(
                "(one d) -> one d", one=1))
            grow = const.tile([1, D], F32R)
            nc.vector.tensor_copy(grow[:], grow_f[:])
            brow_f = const.tile([1, D], F32)
            nc.sync.dma_start(out=brow_f[:], in_=bap.rearrange(
                "(one d) -> one d", one=1))
            brow = const.tile([1, D], F32R)
            nc.vector.tensor_copy(brow[:], brow_f[:])
            return gcol, grow, brow

        g1col, g1row, b1row = load_ln(g1, b1)
        g2col, g2row, b2row = load_ln(g2, b2)

        # ---------- masks ----------
        sm_u8 = const.tile([128, ST], U8)
        nc.sync.dma_start(out=sm_u8[:], in_=sm.rearrange("(i p) -> p i",
                                                         p=128))
        smf = const.tile([128, ST], F32)
        nc.vector.tensor_copy(smf[:], sm_u8[:])
        nsmf = const.tile([128, ST], F32)
        nc.vector.tensor_scalar(out=nsmf[:], in0=sm_u8[:], scalar1=-1.0,
                                scalar2=1.0, op0=AL.mult, op1=AL.add)
        xm_u8 = const.tile([1, LC], U8)
        nc.sync.dma_start(out=xm_u8[:], in_=xm.rearrange("(one l) -> one l",
                                                         one=1))
        nx_row = const.tile([1, LC], F32R)
        nc.vector.tensor_scalar(out=nx_row[:], in0=xm_u8[:], scalar1=-1.0,
                                scalar2=1.0, op0=AL.mult, op1=AL.add)

        # ---------- persistent activations ----------
        xT_f = persist.tile([128, 2, LC], F32)       # true x (final add)
        xr = persist.tile([128, 2, LC], F32R)        # x for mlp rhs
        QT = persist.tile([128, 2, LC], F32R)
        KT = persist.tile([128, 2, S], F32R)
        V1 = persist.tile([128, ST, H, DH + 1], F32R)
        V2 = persist.tile([128, ST, H, DH + 1], F32R)
        OT = persist.tile([128, 2, LC], F32R)
        nxb = persist.tile([DH + 1, LC], F32)
        ct_early = ctx.enter_context(tc.tile_pool(name="ct_early", bufs=1))
        cts_early = []
        for g in range(2):
            t = ct_early.tile([128, 2, LC], F32, name=f"cte_{g}", tag=f"cte{g}")
            nc.sync.dma_start(out=t[:], in_=CT[g * 256:(g + 1) * 256, :]
                              .rearrange("(j p) l -> p j l", p=128))
            cts_early.append(t)

        # ---------- phase 1: projections ----------
        with ExitStack() as c1:
            p1io = c1.enter_context(tc.tile_pool(name="p1io", bufs=1))
            p1ps = c1.enter_context(tc.tile_pool(name="p1ps", bufs=4,
                                                 space="PSUM"))
            xpe_f = p1io.tile([128, 2, LC], F32, tag="xio")
            nc.sync.dma_start(out=xT_f[:], in_=xT.rearrange(
                "(t p) l -> p t l", p=128))
            nc.sync.dma_start(out=xpe_f[:], in_=xpeT.rearrange(
                "(t p) l -> p t l", p=128))
            qin = p1io.tile([128, 2, LC], F32R, tag="qin")
            nc.vector.tensor_add(qin[:], xT_f[:], xpe_f[:])
            nc.vector.tensor_copy(xr[:], xT_f[:])

            # source loads in 512-col chunks; kin = src+srcpe, srcr = round(src)
            kin = p1io.tile([128, 2, S], F32R, tag="kin")
            srcr = p1io.tile([128, 2, S], F32R, tag="srcr")
            for nt in range(4):
                cs = slice(nt * 512, (nt + 1) * 512)
                sf = p1io.tile([128, 2, 512], F32, tag="sio", bufs=2,
                               name=f"sf_{nt}")
                pf = p1io.tile([128, 2, 512], F32, tag="pio", bufs=2,
                               name=f"pf_{nt}")
                nc.sync.dma_start(out=sf[:], in_=srcT[:, cs].rearrange(
                    "(t p) s -> p t s", p=128))
                nc.sync.dma_start(out=pf[:], in_=srcpeT[:, cs].rearrange(
                    "(t p) s -> p t s", p=128))
                nc.vector.tensor_add(kin[:, :, cs], sf[:], pf[:])
                nc.vector.tensor_copy(srcr[:, :, cs], sf[:])

            # Q^T [256, LC]: both mt tiles into one 2-bank psum, one copy
            qps = p1ps.tile([128, 2, LC], F32, tag="qk", bufs=2, name="qps")
            for mt in range(2):
                for kt in range(2):
                    nc.tensor.matmul(qps[:, mt, :],
                                     wq_r[:, kt, mt * 128:(mt + 1) * 128],
                                     qin[:, kt, :], start=(kt == 0),
                                     stop=(kt == 1))
            nc.scalar.activation(QT[:].rearrange("p t l -> p (t l)"),
                                 qps[:].rearrange("p t l -> p (t l)"),
                                 AF.Copy)
            # K^T [256, S]: nt pairs share a 2-bank psum, one copy per pair
            for mt in range(2):
                for np2 in range(2):
                    ps = p1ps.tile([128, 2, 512], F32, tag="qk", bufs=2,
                                   name=f"kps_{mt}_{np2}")
                    for j in range(2):
                        nt = np2 * 2 + j
                        for kt in range(2):
                            nc.tensor.matmul(
                                ps[:, j, :],
                                wk_r[:, kt, mt * 128:(mt + 1) * 128],
                                kin[:, kt, nt * 512:(nt + 1) * 512],
                                start=(kt == 0), stop=(kt == 1))
                    nc.scalar.activation(
                        KT[:, mt, np2 * 1024:(np2 + 1) * 1024],
                        ps[:].rearrange("p j l -> p (j l)"), AF.Copy)
            # V [s, 256] -> vsb (ACT), V1 = V*sm (gpsimd TS), V2 = V - V1
            vsb = p1io.tile([128, ST, D], F32, tag="vsb")
            for ip in range(ST // 2):
                ps = p1ps.tile([128, 2, D], F32, tag="vps", bufs=2,
                               name=f"vps_{ip}")
                for j in range(2):
                    i = ip * 2 + j
                    for kt in range(2):
                        nc.tensor.matmul(
                            ps[:, j, :], srcr[:, kt, i * 128:(i + 1) * 128],
                            wv_r[:, kt, :], start=(kt == 0), stop=(kt == 1))
                nc.scalar.activation(
                    vsb[:, ip * 2:ip * 2 + 2, :]
                    .rearrange("p i d -> p (i d)"),
                    ps[:].rearrange("p j d -> p (j d)"), AF.Copy)
            for i in range(ST):
                vsb_h = vsb[:, i, :].rearrange("p (h d) -> p h d", h=H)
                nc.vector.tensor_scalar(
                    out=V1[:, i, :, 0:DH], in0=vsb_h,
                    scalar1=smf[:, i:i + 1], scalar2=None, op0=AL.mult)
            nc.vector.tensor_sub(
                V2[:, :, :, 0:DH],
                vsb[:].rearrange("p i (h d) -> p i h d", h=H),
                V1[:, :, :, 0:DH])
            # sm / (1-sm) columns, broadcast over heads via 0-step AP
            sm_b = bass.AP(tensor=smf.tensor, offset=smf[:].offset,
                           ap=[smf[:].ap[0], smf[:].ap[1], [0, H], [0, 1]])
            nsm_b = bass.AP(tensor=nsmf.tensor, offset=nsmf[:].offset,
                            ap=[nsmf[:].ap[0], nsmf[:].ap[1], [0, H], [0, 1]])
            nc.vector.tensor_copy(V1[:, :, :, DH:DH + 1], sm_b)
            nc.vector.tensor_copy(V2[:, :, :, DH:DH + 1], nsm_b)

            # nxb [65, LC] via ones-matmul broadcast of nx_row
            psb = p1ps.tile([DH + 1, LC], F32, tag="nxbps", bufs=1)
            nc.tensor.matmul(psb[:], ones_r[0:1, 0:DH + 1], nx_row[:],
                             start=True, stop=True)
            nc.vector.tensor_copy(nxb[:], psb[:])

        # ---------- phase 2: attention ----------
        with ExitStack() as c2:
            sc_ps = c2.enter_context(tc.tile_pool(name="sc_ps", bufs=2,
                                                  space="PSUM"))
            av_ps = c2.enter_context(tc.tile_pool(name="av_ps", bufs=4,
                                                  space="PSUM"))
            p2sb = c2.enter_context(tc.tile_pool(name="p2sb", bufs=2))
            ct_pool = c2.enter_context(tc.tile_pool(name="ct", bufs=1))
            e_pool = c2.enter_context(tc.tile_pool(name="e", bufs=4))
            cts = list(cts_early)
            for g in range(2, 8):
                t = ct_pool.tile([128, 2, LC], F32, name=f"ct_{g}",
                                 tag=f"ct{g}")
                nc.sync.dma_start(out=t[:], in_=CT[g * 256:(g + 1) * 256, :]
                                  .rearrange("(j p) l -> p j l", p=128))
                cts.append(t)

            for hp in range(2):
                pv_ps = []
                for h2 in range(2):
                    pv_ps.append([av_ps.tile([DH + 1, LC], F32, tag="pv",
                                             name=f"pv_{hp}_{h2}_{v}")
                                  for v in range(2)])
                ehalf = {}
                for half in range(2):
                    for h2 in range(2):
                        ehalf[(half, h2)] = e_pool.tile(
                            [128, 8, LC], F32R, tag="eh",
                            name=f"eh_{hp}_{half}_{h2}")
                        ps_lo = h2 * DH
                        for g in range(4):
                            sc = sc_ps.tile([128, 2, LC], F32, tag="sc",
                                            name=f"sc_{hp}_{half}_{g}_{h2}")
                            for j in range(2):
                                i = half * 8 + g * 2 + j
                                nc.tensor.matmul(
                                    sc[:, j, :],
                                    KT[ps_lo:ps_lo + DH, hp,
                                       i * 128:(i + 1) * 128],
                                    QT[ps_lo:ps_lo + DH, hp, :],
                                    start=True, stop=True)
                            i0 = g * 2
                            nc.vector.tensor_mul(
                                ehalf[(half, h2)][:, i0:i0 + 2, :],
                                sc[:],
                                cts[half * 4 + g][:])
                    for h2 in range(2):
                        for ec in range(8):
                            eh_ap = ehalf[(half, h2)][:, ec:ec + 1, :] \
                                .rearrange("p i l -> p (i l)")
                            nc.scalar.activation(eh_ap, eh_ap.bitcast(F32),
                                                 AF.Exp)
                        h = 2 * hp + h2
                        for j in range(8):
                            i = half * 8 + j
                            for v, pv in enumerate(pv_ps[h2]):
                                vt = (V1 if v == 0 else V2)
                                nc.tensor.matmul(
                                    pv[:], vt[:, i, h, :],
                                    ehalf[(half, h2)][:, j, :],
                                    start=(i == 0), stop=(i == ST - 1),
                                    skip_group_check=True)
                # combines
                for h2 in range(2):
                    h = 2 * hp + h2
                    p1t, p2t = pv_ps[h2]
                    tmp = p2sb.tile([DH + 1, LC], F32, tag="tmp")
                    nc.vector.tensor_mul(tmp[:], nxb[:], p2t[:])
                    ocomb = p2sb.tile([DH + 1, LC], F32, tag="oc")
                    nc.vector.tensor_add(ocomb[:], tmp[:], p1t[:])
                    rr = p2sb.tile([128, LC], F32R, tag="rr")
                    nc.vector.reciprocal(rr[DH:DH + 1, :],
                                         ocomb[DH:DH + 1, :])
                    rb = av_ps.tile([DH, LC], F32, tag="pv")
                    nc.tensor.matmul(rb[:], ones_r[DH:DH + 1, 0:DH],
                                     rr[DH:DH + 1, :], start=True, stop=True)
                    otmp = p2sb.tile([DH, LC], F32R, tag="ot")
                    nc.vector.scalar_tensor_tensor(
                        out=otmp[:], in0=ocomb[0:DH, :], scalar=1.0,
                        in1=rb[:], op0=AL.mult, op1=AL.mult)
                    nc.sync.dma_start(out=OT[h2 * DH:(h2 + 1) * DH, hp, :],
                                      in_=otmp[:])

        # ---------- phase 3: merge + LN1 + MLP + LN2 + residual ----------
        with ExitStack() as c3:
            p3ps = c3.enter_context(tc.tile_pool(name="p3ps", bufs=2,
                                                 space="PSUM"))
            p3sb = c3.enter_context(tc.tile_pool(name="p3sb", bufs=2))

            def layernorm_T(msb, gcol, grow, brow, out_r, out_f32=None,
                            add_x=False):
                """msb: [128,2,LC] f32r sbuf (features on partitions).
                Writes normalized result to out_r (f32r) or, if add_x,
                writes out_f32 = LN(msb) + xT_f."""
                msq = p3sb.tile([128, 2, LC], F32R, tag="msq")
                nc.gpsimd.tensor_mul(msq[:], msb[:], msb[:])
                mean_b = p3ps.tile([128, LC], F32, tag="lnps", bufs=4)
                for kt in range(2):
                    nc.tensor.matmul(mean_b[:], i256_r[:], msb[:, kt, :],
                                     start=(kt == 0), stop=(kt == 1))
                ssq = p3ps.tile([1, LC], F32, tag="lnrow", bufs=1)
                for kt in range(2):
                    nc.tensor.matmul(ssq[:], ones_r[:, 0:1], msq[:, kt, :],
                                     start=(kt == 0), stop=(kt == 1))
                mrow = p3sb.tile([1, LC], F32, tag="lnr1")
                nc.vector.tensor_copy(mrow[:], mean_b[0:1, :])
                m2 = p3sb.tile([1, LC], F32, tag="lnr2")
                nc.vector.tensor_mul(m2[:], mrow[:], mrow[:])
                varr = p3sb.tile([1, LC], F32, tag="lnr3")
                nc.vector.scalar_tensor_tensor(
                    out=varr[:], in0=ssq[:], scalar=1.0 / D, in1=m2[:],
                    op0=AL.mult, op1=AL.subtract)
                sd = p3sb.tile([1, LC], F32, tag="lnr4")
                nc.scalar.activation(sd[:], varr[:], AF.Sqrt, bias=eps_t[:])
                rstd = p3sb.tile([1, LC], F32R, tag="lnr5")
                nc.vector.reciprocal(rstd[:], sd[:])
                mr = p3sb.tile([1, LC], F32R, tag="lnr6")
                nc.vector.tensor_mul(mr[:], mrow[:], rstd[:].bitcast(F32))
                ab = p3ps.tile([128, LC], F32, tag="lnps", bufs=4)
                nc.tensor.matmul(ab[:], ones_r[0:1, :], rstd[:],
                                 start=True, stop=True)
                for t in range(2):
                    bfull = p3ps.tile([128, LC], F32, tag="lnps", bufs=4)
                    nc.tensor.matmul(bfull[:],
                                     grow[:, t * 128:(t + 1) * 128], mr[:],
                                     start=True, stop=False)
                    nc.tensor.matmul(bfull[:],
                                     brow[:, t * 128:(t + 1) * 128],
                                     negones_row[:], start=False, stop=True)
                    tmp = p3sb.tile([128, LC], F32, tag="lnt")
                    nc.vector.scalar_tensor_tensor(
                        out=tmp[:], in0=msb[:, t, :], scalar=gcol[:, t:t + 1],
                        in1=ab[:], op0=AL.mult, op1=AL.mult)
                    if add_x:
                        u = p3sb.tile([128, LC], F32, tag="lnu")
                        nc.vector.tensor_sub(u[:], tmp[:], bfull[:])
                        nc.gpsimd.tensor_add(out_f32[:, t, :], u[:],
                                             xT_f[:, t, :])
                    else:
                        nc.vector.tensor_sub(out_r[:, t, :], tmp[:],
                                             bfull[:])

            # merge: M^T = Wm^T @ OT
            msb = p3sb.tile([128, 2, LC], F32R, tag="msb")
            for mt in range(2):
                ps = p3ps.tile([128, LC], F32, tag="mm")
                for kt in range(2):
                    nc.tensor.matmul(ps[:], wm_r[:, kt, mt * 128:(mt + 1) * 128],
                                     OT[:, kt, :], start=(kt == 0),
                                     stop=(kt == 1))
                nc.scalar.activation(msb[:, mt, :], ps[:], AF.Copy)
            msg = p3sb.tile([128, 2, LC], F32R, tag="msg")
            layernorm_T(msb, g1col, g1row, b1row, msg)

            # mlp1 + relu
            relu = p3sb.tile([128, 4, LC], F32R, tag="relu")
            for ft in range(4):
                ps = p3ps.tile([128, LC], F32, tag="mm")
                for kt in range(4):
                    rhs = xr[:, kt, :] if kt < 2 else msg[:, kt - 2, :]
                    nc.tensor.matmul(ps[:],
                                     w1_r[:, kt, ft * 128:(ft + 1) * 128],
                                     rhs, start=(kt == 0), stop=(kt == 3))
                nc.scalar.activation(relu[:, ft, :], ps[:], AF.Relu)
            # mlp2
            m2sb = p3sb.tile([128, 2, LC], F32R, tag="m2sb")
            for mt in range(2):
                ps = p3ps.tile([128, LC], F32, tag="mm")
                for kt in range(4):
                    nc.tensor.matmul(ps[:],
                                     w2_r[:, kt, mt * 128:(mt + 1) * 128],
                                     relu[:, kt, :], start=(kt == 0),
                                     stop=(kt == 3))
                nc.scalar.activation(m2sb[:, mt, :], ps[:], AF.Copy)
            final = p3sb.tile([128, 2, LC], F32, tag="final")
            layernorm_T(m2sb, g2col, g2row, b2row, None, out_f32=final,
                        add_x=True)
            nc.sync.dma_start(out=outT.rearrange("(t p) l -> p t l", p=128),
                              in_=final[:])

    nc.compile()
    return nc


def _get_nc():
    global _CACHED_NC
    if _CACHED_NC is None:
        _CACHED_NC = build_nc()
    return _CACHED_NC


def kernel(x, source, x_pe, source_pe, x_mask, source_mask, compatibility,
           Wq, Wk, Wv, Wmerge, Wmlp1, Wmlp2, ln1_g, ln1_b, ln2_g, ln2_b):
    nc = _get_nc()
    f32 = np.float32
    xT = np.ascontiguousarray(np.asarray(x, f32).transpose(0, 2, 1))
    xpeT = np.ascontiguousarray(np.asarray(x_pe, f32).transpose(0, 2, 1))
    srcT = np.ascontiguousarray(np.asarray(source, f32).transpose(0, 2, 1))
    srcpeT = np.ascontiguousarray(
        np.asarray(source_pe, f32).transpose(0, 2, 1))
    CT = np.ascontiguousarray(
        np.asarray(compatibility, f32).transpose(0, 2, 1))
    xmu = np.asarray(x_mask).astype(np.uint8)
    smu = np.asarray(source_mask).astype(np.uint8)
    weights = {
        "Wq": np.asarray(Wq, f32), "Wk": np.asarray(Wk, f32),
        "Wv": np.asarray(Wv, f32), "Wm": np.asarray(Wmerge, f32),
        "W1": np.asarray(Wmlp1, f32), "W2": np.asarray(Wmlp2, f32),
        "g1": np.asarray(ln1_g, f32), "b1": np.asarray(ln1_b, f32),
        "g2": np.asarray(ln2_g, f32), "b2": np.asarray(ln2_b, f32),
    }
    in_maps = []
    for c in range(NCORES):
        b, l0 = c // (NCORES // BS), (c % (NCORES // BS)) * LC
        sl = slice(l0, l0 + LC)
        in_maps.append({
            "xT": np.ascontiguousarray(xT[b][:, sl]),
            "xpeT": np.ascontiguousarray(xpeT[b][:, sl]),
            "srcT": srcT[b],
            "srcpeT": srcpeT[b],
            "CT": np.ascontiguousarray(CT[b][:, sl]),
            "xm": np.ascontiguousarray(xmu[b][sl]),
            "sm": smu[b],
            **weights,
        })
    res = run_bass_kernel_spmd(nc, in_maps, list(range(NCORES)))
    outT = np.empty((BS, D, L), f32)
    for c in range(NCORES):
        b, l0 = c // (NCORES // BS), (c % (NCORES // BS)) * LC
        outT[b][:, l0:l0 + LC] = res.results[c]["outT"]
    return np.ascontiguousarray(outT.transpose(0, 2, 1))
